# revision 4
# baseline (speedup 1.0000x reference)
"""Bass/Trainium2 kernel for nn_CausalGraphVAE (GCN message passing VAE).

Sharding: adjacency columns (= AnT output rows) split across 8 cores.
Each core: sigmoid+deg over its (4096,512) column block of edge_score,
dis=1/sqrt(deg) stays local, X-side matmuls sharded by node rows +
bf16 AllGather, big matmul A[:,rc]^T @ (D.Ys) per core, gates/latent/
decoder local, second AllGather for decoder, final outputs gathered on
host. All biases in setup_inputs are folded on host into effective gate
biases (H0=0 makes the R gate dead and only the top half of the gate
Linear weights live).
"""
import sys

if "/opt/trn_rl_repo" not in sys.path:
    sys.path.insert(0, "/opt/trn_rl_repo")

import numpy as np
import ml_dtypes

import concourse.bass as bass
import concourse.tile as tile
from concourse import bacc, mybir
from concourse.bass_utils import run_bass_kernel_spmd

NCORES = 8
N = 4096
COLS = N // NCORES          # 512 adjacency columns per core
KT = N // 128               # 32 contraction k-tiles
GSIZE = 8                   # k-tiles per DMA group in the stream
NG = KT // GSIZE            # 4 stream groups
D_IN = 64
D_EMB = 128
HID = 128
LAT = 64
P = 3
YF = P * 2 * HID            # 768 encoder Y features (z|h per period)
F32 = mybir.dt.float32
BF16 = mybir.dt.bfloat16
BF = ml_dtypes.bfloat16

_CACHE = {}


def _build():
    nc = bacc.Bacc("TRN2", debug=False, num_devices=NCORES)
    AF = mybir.ActivationFunctionType

    # ---- kernel I/O ----
    esc = nc.dram_tensor("esc", [128, KT * COLS], F32, kind="ExternalInput")
    xT = nc.dram_tensor("xT", [P, D_IN, COLS], BF16, kind="ExternalInput")
    eeT = nc.dram_tensor("eeT", [P, D_EMB, COLS], BF16, kind="ExternalInput")
    teT = nc.dram_tensor("teT", [P, D_EMB, COLS], BF16, kind="ExternalInput")
    epsT = nc.dram_tensor("epsT", [LAT, COLS], F32, kind="ExternalInput")
    wc_cat = nc.dram_tensor("wc_cat", [D_IN + 2 * HID, 2 * HID], BF16, kind="ExternalInput")
    w_ent = nc.dram_tensor("w_ent", [D_EMB, HID], BF16, kind="ExternalInput")
    w_time = nc.dram_tensor("w_time", [D_EMB, HID], BF16, kind="ExternalInput")
    wl_z = nc.dram_tensor("wl_z", [HID, HID], BF16, kind="ExternalInput")
    wl_h = nc.dram_tensor("wl_h", [HID, HID], BF16, kind="ExternalInput")
    w_mu = nc.dram_tensor("w_mu", [HID, LAT], BF16, kind="ExternalInput")
    w_lv = nc.dram_tensor("w_lv", [HID, LAT], BF16, kind="ExternalInput")
    w_dec = nc.dram_tensor("w_dec", [LAT, HID], BF16, kind="ExternalInput")
    wd_cat = nc.dram_tensor("wd_cat", [HID, 2 * D_IN], BF16, kind="ExternalInput")
    wld_z = nc.dram_tensor("wld_z", [D_IN, D_IN], BF16, kind="ExternalInput")
    wld_h = nc.dram_tensor("wld_h", [D_IN, D_IN], BF16, kind="ExternalInput")
    nblz = nc.dram_tensor("nblz", [HID, 1], F32, kind="ExternalInput")
    blh = nc.dram_tensor("blh", [HID, 1], F32, kind="ExternalInput")
    b_mu = nc.dram_tensor("b_mu", [LAT, 1], F32, kind="ExternalInput")
    b_lv = nc.dram_tensor("b_lv", [LAT, 1], F32, kind="ExternalInput")
    b_dec = nc.dram_tensor("b_dec", [HID, 1], F32, kind="ExternalInput")
    nbldz = nc.dram_tensor("nbldz", [D_IN, 1], F32, kind="ExternalInput")
    bldh = nc.dram_tensor("bldh", [D_IN, 1], F32, kind="ExternalInput")
    probs_in = nc.dram_tensor("probs_in", [128, P], F32, kind="ExternalInput")

    a_out = nc.dram_tensor("a_out", [128, KT * COLS], F32, kind="ExternalOutput")
    mu_out = nc.dram_tensor("mu_out", [LAT, COLS], F32, kind="ExternalOutput")
    lv_out = nc.dram_tensor("lv_out", [LAT, COLS], F32, kind="ExternalOutput")
    rec_out = nc.dram_tensor("rec_out", [D_IN, COLS], F32, kind="ExternalOutput")

    dis_dram = nc.dram_tensor("dis_dram", [1, COLS], F32)

    with tile.TileContext(nc) as tc:
        with (
            tc.tile_pool(name="singles", bufs=1) as sg,
            tc.tile_pool(name="esc_in", bufs=2) as esc_p,
            tc.tile_pool(name="af32", bufs=2) as af32_p,
            tc.tile_pool(name="ys4", bufs=3) as ys4_p,
            tc.tile_pool(name="yd4", bufs=2) as yd4_p,
            tc.tile_pool(name="work", bufs=2) as wk,
            tc.tile_pool(name="gps", bufs=1, space="PSUM") as gps,
            tc.tile_pool(name="mps", bufs=1, space="PSUM") as mps,
            tc.tile_pool(name="dram", bufs=1, space="DRAM") as dr,
        ):
            # ---- small loads ----
            w_ent_t = sg.tile([D_EMB, HID], BF16)
            w_time_t = sg.tile([D_EMB, HID], BF16)
            wc_x_t = sg.tile([D_IN, 2 * HID], BF16)
            wc_e_t = sg.tile([HID, 2 * HID], BF16)
            wc_t_t = sg.tile([HID, 2 * HID], BF16)
            wl_z_t = sg.tile([HID, HID], BF16)
            wl_h_t = sg.tile([HID, HID], BF16)
            w_mu_t = sg.tile([HID, LAT], BF16)
            w_lv_t = sg.tile([HID, LAT], BF16)
            w_dec_t = sg.tile([LAT, HID], BF16)
            wd_cat_t = sg.tile([HID, 2 * D_IN], BF16)
            wld_z_t = sg.tile([D_IN, D_IN], BF16)
            wld_h_t = sg.tile([D_IN, D_IN], BF16)
            nblz_t = sg.tile([HID, 1], F32)
            blh_t = sg.tile([HID, 1], F32)
            b_mu_t = sg.tile([LAT, 1], F32)
            b_lv_t = sg.tile([LAT, 1], F32)
            b_dec_t = sg.tile([HID, 1], F32)
            nbldz_t = sg.tile([D_IN, 1], F32)
            bldh_t = sg.tile([D_IN, 1], F32)
            probs_t = sg.tile([128, P], F32)
            for t, d in (
                (w_ent_t, w_ent), (w_time_t, w_time),
                (wc_x_t, wc_cat[0:D_IN, :]), (wc_e_t, wc_cat[D_IN:D_IN + HID, :]),
                (wc_t_t, wc_cat[D_IN + HID:, :]),
                (wl_z_t, wl_z), (wl_h_t, wl_h), (w_mu_t, w_mu),
                (w_lv_t, w_lv), (w_dec_t, w_dec), (wd_cat_t, wd_cat),
                (wld_z_t, wld_z), (wld_h_t, wld_h), (nblz_t, nblz),
                (blh_t, blh), (b_mu_t, b_mu), (b_lv_t, b_lv),
                (b_dec_t, b_dec), (nbldz_t, nbldz), (bldh_t, bldh),
                (probs_t, probs_in),
            ):
                nc.sync.dma_start(out=t[:], in_=d[:])
            xT_t = sg.tile([D_IN, P * COLS], BF16)
            eeT_t = sg.tile([D_EMB, P * COLS], BF16)
            teT_t = sg.tile([D_EMB, P * COLS], BF16)
            for p in range(P):
                nc.sync.dma_start(out=xT_t[:, p * COLS:(p + 1) * COLS], in_=xT[p])
                nc.sync.dma_start(out=eeT_t[:, p * COLS:(p + 1) * COLS], in_=eeT[p])
                nc.sync.dma_start(out=teT_t[:, p * COLS:(p + 1) * COLS], in_=teT[p])
            epsT_t = sg.tile([LAT, COLS], F32)
            nc.sync.dma_start(out=epsT_t[:], in_=epsT[:])
            ones_t = sg.tile([128, 1], BF16)
            nc.vector.memset(ones_t[:], 1.0)

            # ---- stream: esc -> sigmoid -> A out, bf16 cast, deg ----
            deg_ps = mps.tile([1, COLS], F32, tag="uda", name="deg_ps")
            a_bf = []
            for g in range(NG):
                W = GSIZE * COLS
                esc_t = esc_p.tile([128, W], F32, name="esc_t")
                nc.sync.dma_start(out=esc_t[:], in_=esc[:, g * W:(g + 1) * W])
                sig_t = af32_p.tile([128, W], F32, name="sig_t")
                nc.scalar.activation(out=sig_t[:], in_=esc_t[:], func=AF.Sigmoid)
                nc.sync.dma_start(out=a_out[:, g * W:(g + 1) * W], in_=sig_t[:])
                ab = sg.tile([128, W], BF16, name=f"a_bf{g}")
                nc.vector.tensor_copy(out=ab[:], in_=sig_t[:])
                a_bf.append(ab)
                for kk in range(GSIZE):
                    nc.tensor.matmul(
                        deg_ps[:], ones_t[:], ab[:, kk * COLS:(kk + 1) * COLS],
                        start=(g == 0 and kk == 0), stop=(g == NG - 1 and kk == GSIZE - 1),
                    )

            def a_tile(ki):
                return a_bf[ki // GSIZE][:, (ki % GSIZE) * COLS:(ki % GSIZE + 1) * COLS]

            # ---- dis = 1/sqrt(deg), local node-major + column-broadcast ----
            sq_t = sg.tile([1, COLS], F32)
            nc.scalar.activation(out=sq_t[:], in_=deg_ps[:], func=AF.Sqrt)
            dis_row = sg.tile([1, COLS], F32)
            nc.vector.reciprocal(out=dis_row[:], in_=sq_t[:])
            nc.sync.dma_start(out=dis_dram[:], in_=dis_row[:])
            dis_nm = sg.tile([128, COLS // 128], F32)
            nc.sync.dma_start(
                out=dis_nm[:],
                in_=dis_dram[0, :].rearrange("(m p) -> p m", p=128),
            )
            dis_bc = sg.tile([128, COLS], F32)
            nc.sync.dma_start(
                out=dis_bc[:],
                in_=bass.AP(tensor=dis_dram[:].tensor, offset=0, ap=[[0, 128], [1, COLS]]),
            )

            # ---- ent/tim relu features (feat-major, bf16) ----
            ent_t = sg.tile([HID, P * COLS], BF16)
            tim_t = sg.tile([HID, P * COLS], BF16)
            for p in range(P):
                ps1 = mps.tile([HID, COLS], F32, tag="sp", name="ent_ps")
                nc.tensor.matmul(ps1[:], w_ent_t[:], eeT_t[:, p * COLS:(p + 1) * COLS],
                                 start=True, stop=True)
                nc.scalar.activation(out=ent_t[:, p * COLS:(p + 1) * COLS], in_=ps1[:],
                                     func=AF.Relu)
                ps2 = mps.tile([HID, COLS], F32, tag="sp", name="tim_ps")
                nc.tensor.matmul(ps2[:], w_time_t[:], teT_t[:, p * COLS:(p + 1) * COLS],
                                 start=True, stop=True)
                nc.scalar.activation(out=tim_t[:, p * COLS:(p + 1) * COLS], in_=ps2[:],
                                     func=AF.Relu)

            # ---- local Y shard (node-major), scale rows by dis, AllGather ----
            ag1_in = dr.tile([COLS, YF], BF16)
            ag1_out = dr.tile([N, YF], BF16, addr_space="Shared")
            MT = COLS // 128  # 4 node chunks
            for p in range(P):
                for m in range(MT):
                    msl = slice(m * 128, (m + 1) * 128)
                    psl = slice(p * COLS, p * COLS + COLS)
                    y_ps = mps.tile([128, 2 * HID], F32, tag="sp", name="y_ps")
                    nc.tensor.matmul(y_ps[:], xT_t[:, psl][:, msl], wc_x_t[:],
                                     start=True, stop=False)
                    nc.tensor.matmul(y_ps[:], ent_t[:, psl][:, msl],
                                     wc_e_t[:], start=False, stop=False)
                    nc.tensor.matmul(y_ps[:], tim_t[:, psl][:, msl],
                                     wc_t_t[:], start=False, stop=True)
                    ysc = wk.tile([128, 2 * HID], BF16, name="ysc")
                    nc.vector.tensor_scalar_mul(ysc[:], y_ps[:], dis_nm[:, m:m + 1])
                    nc.sync.dma_start(
                        out=ag1_in[m * 128:(m + 1) * 128, p * 2 * HID:(p + 1) * 2 * HID],
                        in_=ysc[:])
            nc.gpsimd.collective_compute(
                "AllGather", mybir.AluOpType.bypass,
                ins=[ag1_in[:].opt()], outs=[ag1_out[:].opt()],
                replica_groups=[list(range(NCORES))],
            )

            # ---- encoder big matmul: G[ft] += ys_ki[:,ft].T @ a_ki ----
            g_ps = [gps.tile([128, COLS], F32, tag=f"g{ft}", name=f"g_ps{ft}")
                    for ft in range(6)]
            KB = 4  # k-tiles per lhsT DMA batch
            for kb in range(KT // KB):
                ys4 = ys4_p.tile([128, KB, YF], BF16, name="ys4")
                nc.sync.dma_start(
                    out=ys4[:],
                    in_=ag1_out[kb * KB * 128:(kb + 1) * KB * 128, :]
                    .rearrange("(b p) f -> p b f", p=128),
                )
                for kk in range(KB):
                    ki = kb * KB + kk
                    for ft in range(6):
                        nc.tensor.matmul(
                            g_ps[ft][:], ys4[:, kk, ft * 128:(ft + 1) * 128], a_tile(ki),
                            start=(ki == 0), stop=(ki == KT - 1),
                        )

            # ---- encoder gates + Henc ----
            henc_t = sg.tile([HID, COLS], F32)
            for p in range(P):
                gz_sc = wk.tile([128, COLS], BF16, name="gz_sc")
                nc.vector.tensor_mul(gz_sc[:], g_ps[2 * p][:], dis_bc[:])
                u_ps = mps.tile([128, COLS], F32, tag="uda", name="uz_ps")
                nc.tensor.matmul(u_ps[:], wl_z_t[:], gz_sc[:], start=True, stop=True)
                zc_t = wk.tile([HID, COLS], F32, name="zc_t")
                nc.scalar.activation(out=zc_t[:], in_=u_ps[:], func=AF.Sigmoid,
                                     bias=nblz_t[:], scale=-1.0)
                gh_sc = wk.tile([128, COLS], BF16, name="gh_sc")
                nc.vector.tensor_mul(gh_sc[:], g_ps[2 * p + 1][:], dis_bc[:])
                uh_ps = mps.tile([128, COLS], F32, tag="uda", name="uh_ps")
                nc.tensor.matmul(uh_ps[:], wl_h_t[:], gh_sc[:], start=True, stop=True)
                ht_t = wk.tile([HID, COLS], F32, name="ht_t")
                nc.scalar.activation(out=ht_t[:], in_=uh_ps[:], func=AF.Tanh,
                                     bias=blh_t[:])
                zh_t = wk.tile([HID, COLS], F32, name="zh_t")
                nc.vector.tensor_mul(zh_t[:], zc_t[:], ht_t[:])
                if p == 0:
                    nc.vector.tensor_scalar_mul(henc_t[:], zh_t[:], probs_t[:, 0:1])
                else:
                    zhp_t = wk.tile([HID, COLS], F32, name="zhp_t")
                    nc.vector.tensor_scalar_mul(zhp_t[:], zh_t[:], probs_t[:, p:p + 1])
                    nc.vector.tensor_add(henc_t[:], henc_t[:], zhp_t[:])

            # ---- latent head ----
            h_bf = sg.tile([HID, COLS], BF16)
            nc.scalar.activation(out=h_bf[:], in_=henc_t[:], func=AF.Relu)
            mu_ps = mps.tile([LAT, COLS], F32, tag="sp", name="mu_ps")
            nc.tensor.matmul(mu_ps[:], w_mu_t[:], h_bf[:], start=True, stop=True)
            mu_t = sg.tile([LAT, COLS], F32)
            nc.vector.tensor_scalar_add(mu_t[:], mu_ps[:], b_mu_t[:])
            nc.sync.dma_start(out=mu_out[:], in_=mu_t[:])
            lv_ps = mps.tile([LAT, COLS], F32, tag="sp", name="lv_ps")
            nc.tensor.matmul(lv_ps[:], w_lv_t[:], h_bf[:], start=True, stop=True)
            lv_t = sg.tile([LAT, COLS], F32)
            nc.vector.tensor_scalar_add(lv_t[:], lv_ps[:], b_lv_t[:])
            nc.sync.dma_start(out=lv_out[:], in_=lv_t[:])
            std_t = wk.tile([LAT, COLS], F32, name="std_t")
            nc.scalar.activation(out=std_t[:], in_=lv_t[:], func=AF.Exp, scale=0.5)
            es_t = wk.tile([LAT, COLS], F32, name="es_t")
            nc.vector.tensor_mul(es_t[:], epsT_t[:], std_t[:])
            z_bf = sg.tile([LAT, COLS], BF16)
            nc.vector.tensor_add(z_bf[:], mu_t[:], es_t[:])
            d_ps = mps.tile([HID, COLS], F32, tag="sp", name="d_ps")
            nc.tensor.matmul(d_ps[:], w_dec_t[:], z_bf[:], start=True, stop=True)
            d_bf = sg.tile([HID, COLS], BF16)
            nc.vector.tensor_scalar_add(d_bf[:], d_ps[:], b_dec_t[:])

            # ---- decoder Y shard + AllGather ----
            ag2_in = dr.tile([COLS, 2 * D_IN], BF16)
            ag2_out = dr.tile([N, 2 * D_IN], BF16, addr_space="Shared")
            for m in range(MT):
                yd_ps = mps.tile([128, 2 * D_IN], F32, tag="sp", name="yd_ps")
                nc.tensor.matmul(yd_ps[:], d_bf[:, m * 128:(m + 1) * 128], wd_cat_t[:],
                                 start=True, stop=True)
                ydsc = wk.tile([128, 2 * D_IN], BF16, name="ydsc")
                nc.vector.tensor_scalar_mul(ydsc[:], yd_ps[:], dis_nm[:, m:m + 1])
                nc.sync.dma_start(out=ag2_in[m * 128:(m + 1) * 128, :], in_=ydsc[:])
            nc.gpsimd.collective_compute(
                "AllGather", mybir.AluOpType.bypass,
                ins=[ag2_in[:].opt()], outs=[ag2_out[:].opt()],
                replica_groups=[list(range(NCORES))],
            )

            # ---- decoder big matmul (two M=64 gates) + gates + recon ----
            gdz_ps = gps.tile([D_IN, COLS], F32, tag="g0", name="gdz_ps")
            gdh_ps = gps.tile([D_IN, COLS], F32, tag="g1", name="gdh_ps")
            for kb in range(KT // KB):
                yd4 = yd4_p.tile([128, KB, 2 * D_IN], BF16, name="yd4")
                nc.sync.dma_start(
                    out=yd4[:],
                    in_=ag2_out[kb * KB * 128:(kb + 1) * KB * 128, :]
                    .rearrange("(b p) f -> p b f", p=128),
                )
                for kk in range(KB):
                    ki = kb * KB + kk
                    nc.tensor.matmul(gdz_ps[:], yd4[:, kk, 0:D_IN], a_tile(ki),
                                     start=(ki == 0), stop=(ki == KT - 1))
                    nc.tensor.matmul(gdh_ps[:], yd4[:, kk, D_IN:2 * D_IN], a_tile(ki),
                                     start=(ki == 0), stop=(ki == KT - 1))
            gdz_sc = wk.tile([D_IN, COLS], BF16, name="gdz_sc")
            nc.vector.tensor_mul(gdz_sc[:], gdz_ps[:], dis_bc[0:D_IN, :])
            uzd_ps = mps.tile([D_IN, COLS], F32, tag="uda", name="uzd_ps")
            nc.tensor.matmul(uzd_ps[:], wld_z_t[:], gdz_sc[:], start=True, stop=True)
            zcd_t = wk.tile([D_IN, COLS], F32, name="zcd_t")
            nc.scalar.activation(out=zcd_t[:], in_=uzd_ps[:], func=AF.Sigmoid,
                                 bias=nbldz_t[:], scale=-1.0)
            gdh_sc = wk.tile([D_IN, COLS], BF16, name="gdh_sc")
            nc.vector.tensor_mul(gdh_sc[:], gdh_ps[:], dis_bc[0:D_IN, :])
            uhd_ps = mps.tile([D_IN, COLS], F32, tag="uda", name="uhd_ps")
            nc.tensor.matmul(uhd_ps[:], wld_h_t[:], gdh_sc[:], start=True, stop=True)
            htd_t = wk.tile([D_IN, COLS], F32, name="htd_t")
            nc.scalar.activation(out=htd_t[:], in_=uhd_ps[:], func=AF.Tanh,
                                 bias=bldh_t[:])
            prod_t = wk.tile([D_IN, COLS], F32, name="prod_t")
            nc.vector.tensor_mul(prod_t[:], zcd_t[:], htd_t[:])
            rec_t = wk.tile([D_IN, COLS], F32, name="rec_t")
            nc.vector.tensor_scalar_max(rec_t[:], prod_t[:], 0.0)
            nc.sync.dma_start(out=rec_out[:], in_=rec_t[:])

    nc.compile()
    return nc


def _get_nc():
    if "nc" not in _CACHE:
        _CACHE["nc"] = _build()
    return _CACHE["nc"]


def _eps():
    if "eps" not in _CACHE:
        import jax

        with jax.default_device(jax.devices("cpu")[0]):
            e = jax.random.normal(jax.random.key(42), (N, LAT), jax.numpy.float32)
        _CACHE["eps"] = np.asarray(e)
    return _CACHE["eps"]


def _np(v):
    return np.asarray(v, dtype=np.float32)


def make_in_maps(x, entity_emb, time_emb, params):
    x = _np(x)
    ee = _np(entity_emb)
    te = _np(time_emb)
    p = params
    t1, td = p["t1"], p["td"]
    eps = _eps()

    wc_cat = np.concatenate([_np(t1["Wc_z"]), _np(t1["Wc_h"])], axis=1).astype(BF)
    wd_cat = np.concatenate([_np(td["Wc_z"]), _np(td["Wc_h"])], axis=1).astype(BF)
    wl_z = _np(t1["Wl_z"])[:HID].astype(BF)
    wl_h = _np(t1["Wl_h"])[:HID].astype(BF)
    wld_z = _np(td["Wl_z"])[:D_IN].astype(BF)
    wld_h = _np(td["Wl_h"])[:D_IN].astype(BF)
    nblz = -(_np(t1["bc_z"]) @ _np(t1["Wl_z"])[:HID] + _np(t1["bl_z"]))
    blh = _np(t1["bc_h"]) @ _np(t1["Wl_h"])[:HID] + _np(t1["bl_h"])
    nbldz = -(_np(td["bc_z"]) @ _np(td["Wl_z"])[:D_IN] + _np(td["bl_z"]))
    bldh = _np(td["bc_h"]) @ _np(td["Wl_h"])[:D_IN] + _np(td["bl_h"])
    att = _np(p["att1"])
    probs = np.exp(att - att.max())
    probs = (probs / probs.sum()).astype(np.float32)

    shared = {
        "wc_cat": wc_cat, "wd_cat": wd_cat,
        "w_ent": _np(p["W_ent"]).astype(BF), "w_time": _np(p["W_time"]).astype(BF),
        "wl_z": wl_z, "wl_h": wl_h,
        "w_mu": _np(p["W_mu"]).astype(BF), "w_lv": _np(p["W_lv"]).astype(BF),
        "w_dec": _np(p["W_dec"]).astype(BF),
        "wld_z": wld_z, "wld_h": wld_h,
        "nblz": nblz.reshape(HID, 1).astype(np.float32),
        "blh": blh.reshape(HID, 1).astype(np.float32),
        "b_mu": _np(p["b_mu"]).reshape(LAT, 1),
        "b_lv": _np(p["b_lv"]).reshape(LAT, 1),
        "b_dec": _np(p["b_dec"]).reshape(HID, 1),
        "nbldz": nbldz.reshape(D_IN, 1).astype(np.float32),
        "bldh": bldh.reshape(D_IN, 1).astype(np.float32),
        "probs_in": np.broadcast_to(probs, (128, P)).copy(),
    }

    es_full = _np(params["edge_score"])
    in_maps = []
    for c in range(NCORES):
        rc = slice(c * COLS, (c + 1) * COLS)
        blk = es_full[:, rc]  # (4096, 512)
        esc_tiled = np.ascontiguousarray(
            blk.reshape(KT, 128, COLS).transpose(1, 0, 2).reshape(128, KT * COLS))
        m = dict(shared)
        m["esc"] = esc_tiled
        m["xT"] = np.ascontiguousarray(x[rc].transpose(1, 2, 0)).astype(BF)
        m["eeT"] = np.ascontiguousarray(ee[rc].transpose(1, 2, 0)).astype(BF)
        m["teT"] = np.ascontiguousarray(te[rc].transpose(1, 2, 0)).astype(BF)
        m["epsT"] = np.ascontiguousarray(eps[rc].T)
        in_maps.append(m)
    return in_maps


def assemble(results):
    a_blocks = []
    mu_blocks, lv_blocks, rec_blocks = [], [], []
    for c in range(NCORES):
        r = results[c]
        a_blocks.append(
            r["a_out"].reshape(128, KT, COLS).transpose(1, 0, 2).reshape(N, COLS))
        mu_blocks.append(r["mu_out"].T)
        lv_blocks.append(r["lv_out"].T)
        rec_blocks.append(r["rec_out"].T)
    A = np.concatenate(a_blocks, axis=1)
    mu = np.concatenate(mu_blocks, axis=0)
    lv = np.concatenate(lv_blocks, axis=0)
    rec = np.concatenate(rec_blocks, axis=0)
    return rec, mu, lv, A


def kernel(x, entity_emb, time_emb, num_nodes, params):
    nc = _get_nc()
    in_maps = make_in_maps(x, entity_emb, time_emb, params)
    res = run_bass_kernel_spmd(nc, in_maps, list(range(NCORES)))
    return assemble(res.results)


# revision 5
# speedup vs baseline: 1.0604x; 1.0604x over previous
"""Bass/Trainium2 kernel for nn_CausalGraphVAE (GCN message passing VAE).

Sharding: adjacency columns (= AnT output rows) split across 8 cores.
Each core: sigmoid+deg over its (4096,512) column block of edge_score,
dis=1/sqrt(deg) stays local, X-side matmuls sharded by node rows +
bf16 AllGather, big matmul A[:,rc]^T @ (D.Ys) per core, gates/latent/
decoder local, second AllGather for decoder, final outputs gathered on
host. All biases in setup_inputs are folded on host into effective gate
biases (H0=0 makes the R gate dead and only the top half of the gate
Linear weights live).
"""
import sys

if "/opt/trn_rl_repo" not in sys.path:
    sys.path.insert(0, "/opt/trn_rl_repo")

import numpy as np
import ml_dtypes

import concourse.bass as bass
import concourse.tile as tile
from concourse import bacc, mybir
from concourse.bass_utils import run_bass_kernel_spmd

NCORES = 8
N = 4096
COLS = N // NCORES          # 512 adjacency columns per core
KT = N // 128               # 32 contraction k-tiles
GSIZE = 8                   # k-tiles per DMA group in the stream
NG = KT // GSIZE            # 4 stream groups
D_IN = 64
D_EMB = 128
HID = 128
LAT = 64
P = 3
YF = P * 2 * HID            # 768 encoder Y features (z|h per period)
F32 = mybir.dt.float32
BF16 = mybir.dt.bfloat16
BF = ml_dtypes.bfloat16

_CACHE = {}


def _build():
    nc = bacc.Bacc("TRN2", debug=False, num_devices=NCORES)
    AF = mybir.ActivationFunctionType

    # ---- kernel I/O ----
    esc = nc.dram_tensor("esc", [128, KT * COLS], F32, kind="ExternalInput")
    xT = nc.dram_tensor("xT", [P, D_IN, COLS], BF16, kind="ExternalInput")
    eeT = nc.dram_tensor("eeT", [P, D_EMB, COLS], BF16, kind="ExternalInput")
    teT = nc.dram_tensor("teT", [P, D_EMB, COLS], BF16, kind="ExternalInput")
    epsT = nc.dram_tensor("epsT", [LAT, COLS], F32, kind="ExternalInput")
    wc_cat = nc.dram_tensor("wc_cat", [D_IN + 2 * HID, 2 * HID], BF16, kind="ExternalInput")
    w_ent = nc.dram_tensor("w_ent", [D_EMB, HID], BF16, kind="ExternalInput")
    w_time = nc.dram_tensor("w_time", [D_EMB, HID], BF16, kind="ExternalInput")
    wl_z = nc.dram_tensor("wl_z", [HID, HID], BF16, kind="ExternalInput")
    wl_h = nc.dram_tensor("wl_h", [HID, HID], BF16, kind="ExternalInput")
    w_mu = nc.dram_tensor("w_mu", [HID, LAT], BF16, kind="ExternalInput")
    w_lv = nc.dram_tensor("w_lv", [HID, LAT], BF16, kind="ExternalInput")
    w_dec = nc.dram_tensor("w_dec", [LAT, HID], BF16, kind="ExternalInput")
    wd_cat = nc.dram_tensor("wd_cat", [HID, 2 * D_IN], BF16, kind="ExternalInput")
    wld_z = nc.dram_tensor("wld_z", [D_IN, D_IN], BF16, kind="ExternalInput")
    wld_h = nc.dram_tensor("wld_h", [D_IN, D_IN], BF16, kind="ExternalInput")
    nblz = nc.dram_tensor("nblz", [HID, 1], F32, kind="ExternalInput")
    blh = nc.dram_tensor("blh", [HID, 1], F32, kind="ExternalInput")
    b_mu = nc.dram_tensor("b_mu", [LAT, 1], F32, kind="ExternalInput")
    b_lv = nc.dram_tensor("b_lv", [LAT, 1], F32, kind="ExternalInput")
    b_dec = nc.dram_tensor("b_dec", [HID, 1], F32, kind="ExternalInput")
    nbldz = nc.dram_tensor("nbldz", [D_IN, 1], F32, kind="ExternalInput")
    bldh = nc.dram_tensor("bldh", [D_IN, 1], F32, kind="ExternalInput")
    probs_in = nc.dram_tensor("probs_in", [128, P], F32, kind="ExternalInput")

    a_out = nc.dram_tensor("a_out", [128, KT * COLS], F32, kind="ExternalOutput")
    mu_out = nc.dram_tensor("mu_out", [LAT, COLS], F32, kind="ExternalOutput")
    lv_out = nc.dram_tensor("lv_out", [LAT, COLS], F32, kind="ExternalOutput")
    rec_out = nc.dram_tensor("rec_out", [D_IN, COLS], F32, kind="ExternalOutput")

    dis_dram = nc.dram_tensor("dis_dram", [1, COLS], F32)
    dis_full = nc.dram_tensor("dis_full", [N], F32, addr_space="Shared")

    with tile.TileContext(nc) as tc:
        with (
            tc.tile_pool(name="singles", bufs=1) as sg,
            tc.tile_pool(name="esc_in", bufs=4) as esc_p,
            tc.tile_pool(name="ys4", bufs=3) as ys4_p,
            tc.tile_pool(name="yd4", bufs=2) as yd4_p,
            tc.tile_pool(name="work", bufs=2) as wk,
            tc.tile_pool(name="gps", bufs=1, space="PSUM") as gps,
            tc.tile_pool(name="mps", bufs=1, space="PSUM") as mps,
            tc.tile_pool(name="dram", bufs=1, space="DRAM") as dr,
        ):
            # ---- stream first: esc -> sigmoid(in place) -> deg (f32) ----
            ones_t = sg.tile([128, 1], F32)
            nc.vector.memset(ones_t[:], 1.0)
            ones_row = sg.tile([1, 128], F32)
            nc.vector.memset(ones_row[:], 1.0)
            deg_ps = mps.tile([1, COLS], F32, tag="uda", name="deg_ps")
            esc_ts = []
            W = GSIZE * COLS
            for g in range(NG):
                esc_t = esc_p.tile([128, W], F32, name="esc_t")
                nc.sync.dma_start(out=esc_t[:], in_=esc[:, g * W:(g + 1) * W])
                nc.scalar.activation(out=esc_t[:], in_=esc_t[:], func=AF.Sigmoid)
                esc_ts.append(esc_t)
                for kk in range(GSIZE):
                    nc.tensor.matmul(
                        deg_ps[:], ones_t[:], esc_t[:, kk * COLS:(kk + 1) * COLS],
                        start=(g == 0 and kk == 0), stop=(g == NG - 1 and kk == GSIZE - 1),
                    )

            # ---- small loads ----
            w_ent_t = sg.tile([D_EMB, HID], BF16)
            w_time_t = sg.tile([D_EMB, HID], BF16)
            wc_x_t = sg.tile([D_IN, 2 * HID], BF16)
            wc_e_t = sg.tile([HID, 2 * HID], BF16)
            wc_t_t = sg.tile([HID, 2 * HID], BF16)
            wl_z_t = sg.tile([HID, HID], BF16)
            wl_h_t = sg.tile([HID, HID], BF16)
            w_mu_t = sg.tile([HID, LAT], BF16)
            w_lv_t = sg.tile([HID, LAT], BF16)
            w_dec_t = sg.tile([LAT, HID], BF16)
            wd_cat_t = sg.tile([HID, 2 * D_IN], BF16)
            wld_z_t = sg.tile([D_IN, D_IN], BF16)
            wld_h_t = sg.tile([D_IN, D_IN], BF16)
            nblz_t = sg.tile([HID, 1], F32)
            blh_t = sg.tile([HID, 1], F32)
            b_mu_t = sg.tile([LAT, 1], F32)
            b_lv_t = sg.tile([LAT, 1], F32)
            b_dec_t = sg.tile([HID, 1], F32)
            nbldz_t = sg.tile([D_IN, 1], F32)
            bldh_t = sg.tile([D_IN, 1], F32)
            probs_t = sg.tile([128, P], F32)
            for t, d in (
                (w_ent_t, w_ent), (w_time_t, w_time),
                (wc_x_t, wc_cat[0:D_IN, :]), (wc_e_t, wc_cat[D_IN:D_IN + HID, :]),
                (wc_t_t, wc_cat[D_IN + HID:, :]),
                (wl_z_t, wl_z), (wl_h_t, wl_h), (w_mu_t, w_mu),
                (w_lv_t, w_lv), (w_dec_t, w_dec), (wd_cat_t, wd_cat),
                (wld_z_t, wld_z), (wld_h_t, wld_h), (nblz_t, nblz),
                (blh_t, blh), (b_mu_t, b_mu), (b_lv_t, b_lv),
                (b_dec_t, b_dec), (nbldz_t, nbldz), (bldh_t, bldh),
                (probs_t, probs_in),
            ):
                nc.sync.dma_start(out=t[:], in_=d[:])
            xT_t = sg.tile([D_IN, P * COLS], BF16)
            eeT_t = sg.tile([D_EMB, P * COLS], BF16)
            teT_t = sg.tile([D_EMB, P * COLS], BF16)
            for p in range(P):
                nc.sync.dma_start(out=xT_t[:, p * COLS:(p + 1) * COLS], in_=xT[p])
                nc.sync.dma_start(out=eeT_t[:, p * COLS:(p + 1) * COLS], in_=eeT[p])
                nc.sync.dma_start(out=teT_t[:, p * COLS:(p + 1) * COLS], in_=teT[p])
            epsT_t = sg.tile([LAT, COLS], F32)
            nc.sync.dma_start(out=epsT_t[:], in_=epsT[:])

            # ---- ent/tim relu features (feat-major, bf16) ----
            ent_t = sg.tile([HID, P * COLS], BF16)
            tim_t = sg.tile([HID, P * COLS], BF16)
            for p in range(P):
                ps1 = gps.tile([HID, COLS], F32, tag="g2", name="ent_ps")
                nc.tensor.matmul(ps1[:], w_ent_t[:], eeT_t[:, p * COLS:(p + 1) * COLS],
                                 start=True, stop=True)
                nc.scalar.activation(out=ent_t[:, p * COLS:(p + 1) * COLS], in_=ps1[:],
                                     func=AF.Relu)
                ps2 = gps.tile([HID, COLS], F32, tag="g3", name="tim_ps")
                nc.tensor.matmul(ps2[:], w_time_t[:], teT_t[:, p * COLS:(p + 1) * COLS],
                                 start=True, stop=True)
                nc.scalar.activation(out=tim_t[:, p * COLS:(p + 1) * COLS], in_=ps2[:],
                                     func=AF.Relu)

            # ---- local Y shard (node-major, unscaled), AllGather early ----
            ag1_in = dr.tile([COLS, YF], BF16)
            ag1_out = dr.tile([N, YF], BF16, addr_space="Shared")
            MT = COLS // 128  # 4 node chunks
            for p in range(P):
                for m in range(MT):
                    msl = slice(m * 128, (m + 1) * 128)
                    psl = slice(p * COLS, p * COLS + COLS)
                    y_ps = gps.tile([128, 2 * HID], F32, tag=f"g{m % 2}", name="y_ps")
                    nc.tensor.matmul(y_ps[:], xT_t[:, psl][:, msl], wc_x_t[:],
                                     start=True, stop=False)
                    nc.tensor.matmul(y_ps[:], ent_t[:, psl][:, msl],
                                     wc_e_t[:], start=False, stop=False)
                    nc.tensor.matmul(y_ps[:], tim_t[:, psl][:, msl],
                                     wc_t_t[:], start=False, stop=True)
                    ysc = wk.tile([128, 2 * HID], BF16, name="ysc")
                    nc.vector.tensor_copy(out=ysc[:], in_=y_ps[:])
                    nc.sync.dma_start(
                        out=ag1_in[m * 128:(m + 1) * 128, p * 2 * HID:(p + 1) * 2 * HID],
                        in_=ysc[:])
            nc.gpsimd.collective_compute(
                "AllGather", mybir.AluOpType.bypass,
                ins=[ag1_in[:].opt()], outs=[ag1_out[:].opt()],
                replica_groups=[list(range(NCORES))],
            )

            # ---- dis = 1/sqrt(deg) broadcast to [128, COLS]; AllGather dis ----
            deg_sb = sg.tile([1, COLS], F32)
            nc.vector.tensor_copy(out=deg_sb[:], in_=deg_ps[:])
            bc_ps = mps.tile([128, COLS], F32, tag="sp", name="bc_ps")
            nc.tensor.matmul(bc_ps[:], ones_row[:], deg_sb[:], start=True, stop=True)
            sq_bc = sg.tile([128, COLS], F32)
            nc.scalar.activation(out=sq_bc[:], in_=bc_ps[:], func=AF.Sqrt)
            dis_bc = sg.tile([128, COLS], F32)
            nc.vector.reciprocal(out=dis_bc[:], in_=sq_bc[:])
            nc.sync.dma_start(out=dis_dram[:], in_=dis_bc[0:1, :])
            nc.gpsimd.collective_compute(
                "AllGather", mybir.AluOpType.bypass,
                ins=[dis_dram[:].opt()], outs=[dis_full[:].opt()],
                replica_groups=[list(range(NCORES))],
            )
            disf_nm = sg.tile([128, KT], F32)
            nc.sync.dma_start(
                out=disf_nm[:],
                in_=dis_full[:].rearrange("(k p) -> p k", p=128),
            )

            # ---- deferred: A output writes + bf16 casts + dis_i scale in place ----
            a_bf = []
            for g in range(NG):
                nc.sync.dma_start(out=a_out[:, g * W:(g + 1) * W], in_=esc_ts[g][:])
                ab = sg.tile([128, W], BF16, name=f"a_bf{g}")
                nc.vector.tensor_copy(out=ab[:], in_=esc_ts[g][:])
                a_bf.append(ab)

            def a_tile(ki):
                return a_bf[ki // GSIZE][:, (ki % GSIZE) * COLS:(ki % GSIZE + 1) * COLS]

            for ki in range(KT):
                at = a_tile(ki)
                nc.vector.tensor_scalar_mul(at, at, disf_nm[:, ki:ki + 1])

            # ---- encoder big matmul: G[ft] += ys_ki[:,ft].T @ a_ki ----
            g_ps = [gps.tile([128, COLS], F32, tag=f"g{ft}", name=f"g_ps{ft}")
                    for ft in range(6)]
            KB = 4  # k-tiles per lhsT DMA batch
            for kb in range(KT // KB):
                ys4 = ys4_p.tile([128, KB, YF], BF16, name="ys4")
                nc.sync.dma_start(
                    out=ys4[:],
                    in_=ag1_out[kb * KB * 128:(kb + 1) * KB * 128, :]
                    .rearrange("(b p) f -> p b f", p=128),
                )
                for kk in range(KB):
                    ki = kb * KB + kk
                    for ft in range(6):
                        nc.tensor.matmul(
                            g_ps[ft][:], ys4[:, kk, ft * 128:(ft + 1) * 128], a_tile(ki),
                            start=(ki == 0), stop=(ki == KT - 1),
                        )

            # ---- encoder gates + Henc ----
            henc_t = sg.tile([HID, COLS], F32)
            for p in range(P):
                gz_sc = wk.tile([128, COLS], BF16, name="gz_sc")
                nc.vector.tensor_mul(gz_sc[:], g_ps[2 * p][:], dis_bc[:])
                u_ps = mps.tile([128, COLS], F32, tag="uda", name="uz_ps")
                nc.tensor.matmul(u_ps[:], wl_z_t[:], gz_sc[:], start=True, stop=True)
                zc_t = wk.tile([HID, COLS], F32, name="zc_t")
                nc.scalar.activation(out=zc_t[:], in_=u_ps[:], func=AF.Sigmoid,
                                     bias=nblz_t[:], scale=-1.0)
                gh_sc = wk.tile([128, COLS], BF16, name="gh_sc")
                nc.vector.tensor_mul(gh_sc[:], g_ps[2 * p + 1][:], dis_bc[:])
                uh_ps = mps.tile([128, COLS], F32, tag="uda", name="uh_ps")
                nc.tensor.matmul(uh_ps[:], wl_h_t[:], gh_sc[:], start=True, stop=True)
                ht_t = wk.tile([HID, COLS], F32, name="ht_t")
                nc.scalar.activation(out=ht_t[:], in_=uh_ps[:], func=AF.Tanh,
                                     bias=blh_t[:])
                zh_t = wk.tile([HID, COLS], F32, name="zh_t")
                nc.vector.tensor_mul(zh_t[:], zc_t[:], ht_t[:])
                if p == 0:
                    nc.vector.tensor_scalar_mul(henc_t[:], zh_t[:], probs_t[:, 0:1])
                else:
                    zhp_t = wk.tile([HID, COLS], F32, name="zhp_t")
                    nc.vector.tensor_scalar_mul(zhp_t[:], zh_t[:], probs_t[:, p:p + 1])
                    nc.vector.tensor_add(henc_t[:], henc_t[:], zhp_t[:])

            # ---- latent head ----
            h_bf = sg.tile([HID, COLS], BF16)
            nc.scalar.activation(out=h_bf[:], in_=henc_t[:], func=AF.Relu)
            mu_ps = mps.tile([LAT, COLS], F32, tag="sp", name="mu_ps")
            nc.tensor.matmul(mu_ps[:], w_mu_t[:], h_bf[:], start=True, stop=True)
            mu_t = sg.tile([LAT, COLS], F32)
            nc.vector.tensor_scalar_add(mu_t[:], mu_ps[:], b_mu_t[:])
            nc.sync.dma_start(out=mu_out[:], in_=mu_t[:])
            lv_ps = mps.tile([LAT, COLS], F32, tag="sp", name="lv_ps")
            nc.tensor.matmul(lv_ps[:], w_lv_t[:], h_bf[:], start=True, stop=True)
            lv_t = sg.tile([LAT, COLS], F32)
            nc.vector.tensor_scalar_add(lv_t[:], lv_ps[:], b_lv_t[:])
            nc.sync.dma_start(out=lv_out[:], in_=lv_t[:])
            std_t = wk.tile([LAT, COLS], F32, name="std_t")
            nc.scalar.activation(out=std_t[:], in_=lv_t[:], func=AF.Exp, scale=0.5)
            es_t = wk.tile([LAT, COLS], F32, name="es_t")
            nc.vector.tensor_mul(es_t[:], epsT_t[:], std_t[:])
            z_bf = sg.tile([LAT, COLS], BF16)
            nc.vector.tensor_add(z_bf[:], mu_t[:], es_t[:])
            d_ps = mps.tile([HID, COLS], F32, tag="sp", name="d_ps")
            nc.tensor.matmul(d_ps[:], w_dec_t[:], z_bf[:], start=True, stop=True)
            d_bf = sg.tile([HID, COLS], BF16)
            nc.vector.tensor_scalar_add(d_bf[:], d_ps[:], b_dec_t[:])

            # ---- decoder Y shard (unscaled) + AllGather ----
            ag2_in = dr.tile([COLS, 2 * D_IN], BF16)
            ag2_out = dr.tile([N, 2 * D_IN], BF16, addr_space="Shared")
            for m in range(MT):
                yd_ps = gps.tile([128, 2 * D_IN], F32, tag=f"g{2 + m % 2}", name="yd_ps")
                nc.tensor.matmul(yd_ps[:], d_bf[:, m * 128:(m + 1) * 128], wd_cat_t[:],
                                 start=True, stop=True)
                ydsc = wk.tile([128, 2 * D_IN], BF16, name="ydsc")
                nc.vector.tensor_copy(out=ydsc[:], in_=yd_ps[:])
                nc.sync.dma_start(out=ag2_in[m * 128:(m + 1) * 128, :], in_=ydsc[:])
            nc.gpsimd.collective_compute(
                "AllGather", mybir.AluOpType.bypass,
                ins=[ag2_in[:].opt()], outs=[ag2_out[:].opt()],
                replica_groups=[list(range(NCORES))],
            )

            # ---- decoder big matmul (two M=64 gates) + gates + recon ----
            gdz_ps = gps.tile([D_IN, COLS], F32, tag="g0", name="gdz_ps")
            gdh_ps = gps.tile([D_IN, COLS], F32, tag="g1", name="gdh_ps")
            for kb in range(KT // KB):
                yd4 = yd4_p.tile([128, KB, 2 * D_IN], BF16, name="yd4")
                nc.sync.dma_start(
                    out=yd4[:],
                    in_=ag2_out[kb * KB * 128:(kb + 1) * KB * 128, :]
                    .rearrange("(b p) f -> p b f", p=128),
                )
                for kk in range(KB):
                    ki = kb * KB + kk
                    nc.tensor.matmul(gdz_ps[:], yd4[:, kk, 0:D_IN], a_tile(ki),
                                     start=(ki == 0), stop=(ki == KT - 1))
                    nc.tensor.matmul(gdh_ps[:], yd4[:, kk, D_IN:2 * D_IN], a_tile(ki),
                                     start=(ki == 0), stop=(ki == KT - 1))
            gdz_sc = wk.tile([D_IN, COLS], BF16, name="gdz_sc")
            nc.vector.tensor_mul(gdz_sc[:], gdz_ps[:], dis_bc[0:D_IN, :])
            uzd_ps = mps.tile([D_IN, COLS], F32, tag="uda", name="uzd_ps")
            nc.tensor.matmul(uzd_ps[:], wld_z_t[:], gdz_sc[:], start=True, stop=True)
            zcd_t = wk.tile([D_IN, COLS], F32, name="zcd_t")
            nc.scalar.activation(out=zcd_t[:], in_=uzd_ps[:], func=AF.Sigmoid,
                                 bias=nbldz_t[:], scale=-1.0)
            gdh_sc = wk.tile([D_IN, COLS], BF16, name="gdh_sc")
            nc.vector.tensor_mul(gdh_sc[:], gdh_ps[:], dis_bc[0:D_IN, :])
            uhd_ps = mps.tile([D_IN, COLS], F32, tag="uda", name="uhd_ps")
            nc.tensor.matmul(uhd_ps[:], wld_h_t[:], gdh_sc[:], start=True, stop=True)
            htd_t = wk.tile([D_IN, COLS], F32, name="htd_t")
            nc.scalar.activation(out=htd_t[:], in_=uhd_ps[:], func=AF.Tanh,
                                 bias=bldh_t[:])
            prod_t = wk.tile([D_IN, COLS], F32, name="prod_t")
            nc.vector.tensor_mul(prod_t[:], zcd_t[:], htd_t[:])
            rec_t = wk.tile([D_IN, COLS], F32, name="rec_t")
            nc.vector.tensor_scalar_max(rec_t[:], prod_t[:], 0.0)
            nc.sync.dma_start(out=rec_out[:], in_=rec_t[:])

    nc.compile()
    return nc


def _get_nc():
    if "nc" not in _CACHE:
        _CACHE["nc"] = _build()
    return _CACHE["nc"]


def _eps():
    if "eps" not in _CACHE:
        import jax

        with jax.default_device(jax.devices("cpu")[0]):
            e = jax.random.normal(jax.random.key(42), (N, LAT), jax.numpy.float32)
        _CACHE["eps"] = np.asarray(e)
    return _CACHE["eps"]


def _np(v):
    return np.asarray(v, dtype=np.float32)


def make_in_maps(x, entity_emb, time_emb, params):
    x = _np(x)
    ee = _np(entity_emb)
    te = _np(time_emb)
    p = params
    t1, td = p["t1"], p["td"]
    eps = _eps()

    wc_cat = np.concatenate([_np(t1["Wc_z"]), _np(t1["Wc_h"])], axis=1).astype(BF)
    wd_cat = np.concatenate([_np(td["Wc_z"]), _np(td["Wc_h"])], axis=1).astype(BF)
    wl_z = _np(t1["Wl_z"])[:HID].astype(BF)
    wl_h = _np(t1["Wl_h"])[:HID].astype(BF)
    wld_z = _np(td["Wl_z"])[:D_IN].astype(BF)
    wld_h = _np(td["Wl_h"])[:D_IN].astype(BF)
    nblz = -(_np(t1["bc_z"]) @ _np(t1["Wl_z"])[:HID] + _np(t1["bl_z"]))
    blh = _np(t1["bc_h"]) @ _np(t1["Wl_h"])[:HID] + _np(t1["bl_h"])
    nbldz = -(_np(td["bc_z"]) @ _np(td["Wl_z"])[:D_IN] + _np(td["bl_z"]))
    bldh = _np(td["bc_h"]) @ _np(td["Wl_h"])[:D_IN] + _np(td["bl_h"])
    att = _np(p["att1"])
    probs = np.exp(att - att.max())
    probs = (probs / probs.sum()).astype(np.float32)

    shared = {
        "wc_cat": wc_cat, "wd_cat": wd_cat,
        "w_ent": _np(p["W_ent"]).astype(BF), "w_time": _np(p["W_time"]).astype(BF),
        "wl_z": wl_z, "wl_h": wl_h,
        "w_mu": _np(p["W_mu"]).astype(BF), "w_lv": _np(p["W_lv"]).astype(BF),
        "w_dec": _np(p["W_dec"]).astype(BF),
        "wld_z": wld_z, "wld_h": wld_h,
        "nblz": nblz.reshape(HID, 1).astype(np.float32),
        "blh": blh.reshape(HID, 1).astype(np.float32),
        "b_mu": _np(p["b_mu"]).reshape(LAT, 1),
        "b_lv": _np(p["b_lv"]).reshape(LAT, 1),
        "b_dec": _np(p["b_dec"]).reshape(HID, 1),
        "nbldz": nbldz.reshape(D_IN, 1).astype(np.float32),
        "bldh": bldh.reshape(D_IN, 1).astype(np.float32),
        "probs_in": np.broadcast_to(probs, (128, P)).copy(),
    }

    es_full = _np(params["edge_score"])
    in_maps = []
    for c in range(NCORES):
        rc = slice(c * COLS, (c + 1) * COLS)
        blk = es_full[:, rc]  # (4096, 512)
        esc_tiled = np.ascontiguousarray(
            blk.reshape(KT, 128, COLS).transpose(1, 0, 2).reshape(128, KT * COLS))
        m = dict(shared)
        m["esc"] = esc_tiled
        m["xT"] = np.ascontiguousarray(x[rc].transpose(1, 2, 0)).astype(BF)
        m["eeT"] = np.ascontiguousarray(ee[rc].transpose(1, 2, 0)).astype(BF)
        m["teT"] = np.ascontiguousarray(te[rc].transpose(1, 2, 0)).astype(BF)
        m["epsT"] = np.ascontiguousarray(eps[rc].T)
        in_maps.append(m)
    return in_maps


def assemble(results):
    a_blocks = []
    mu_blocks, lv_blocks, rec_blocks = [], [], []
    for c in range(NCORES):
        r = results[c]
        a_blocks.append(
            r["a_out"].reshape(128, KT, COLS).transpose(1, 0, 2).reshape(N, COLS))
        mu_blocks.append(r["mu_out"].T)
        lv_blocks.append(r["lv_out"].T)
        rec_blocks.append(r["rec_out"].T)
    A = np.concatenate(a_blocks, axis=1)
    mu = np.concatenate(mu_blocks, axis=0)
    lv = np.concatenate(lv_blocks, axis=0)
    rec = np.concatenate(rec_blocks, axis=0)
    return rec, mu, lv, A


def kernel(x, entity_emb, time_emb, num_nodes, params):
    nc = _get_nc()
    in_maps = make_in_maps(x, entity_emb, time_emb, params)
    res = run_bass_kernel_spmd(nc, in_maps, list(range(NCORES)))
    return assemble(res.results)


# revision 6
# speedup vs baseline: 1.0720x; 1.0110x over previous
"""Bass/Trainium2 kernel for nn_CausalGraphVAE (GCN message passing VAE).

Sharding: adjacency columns (= AnT output rows) split across 8 cores.
Per core: sigmoid+deg over its (4096,512) column block of edge_score
(in-place sigmoid, deferred A writes), dis=1/sqrt(deg) via broadcast
matmul, tiny dis AllGather, dis_i folded into the resident bf16 a-tiles,
X-side matmuls sharded by node rows with an early unscaled bf16
AllGather, big matmul A[:,rc]^T @ Ys per core, gates/latent local
(H0=0 kills the R gate; gate biases folded on host), second AllGather
for the decoder, outputs gathered on host. DMA traffic is split across
the two HWDGE rings: sync carries the big edge_score stream + A writes,
scalar carries everything else.
"""
import sys

if "/opt/trn_rl_repo" not in sys.path:
    sys.path.insert(0, "/opt/trn_rl_repo")

import numpy as np
import ml_dtypes

import concourse.bass as bass
import concourse.tile as tile
from concourse import bacc, mybir
from concourse.bass_utils import run_bass_kernel_spmd

NCORES = 8
N = 4096
COLS = N // NCORES          # 512 adjacency columns per core
KT = N // 128               # 32 contraction k-tiles
GSIZE = 8                   # k-tiles per DMA group in the stream
NG = KT // GSIZE            # 4 stream groups
D_IN = 64
D_EMB = 128
HID = 128
LAT = 64
P = 3
YF = P * 2 * HID            # 768 encoder Y features (z|h per period)
KB = 4                      # k-tiles per lhsT DMA batch in big matmuls
F32 = mybir.dt.float32
BF16 = mybir.dt.bfloat16
BF = ml_dtypes.bfloat16

# weight blob layout: name -> (offset, partitions, cols)
WSPEC = [
    ("w_ent", D_EMB, HID), ("w_time", D_EMB, HID),
    ("wc_x", D_IN, 2 * HID), ("wc_e", HID, 2 * HID), ("wc_t", HID, 2 * HID),
    ("wl_z", HID, HID), ("wl_h", HID, HID),
    ("w_mu", HID, LAT), ("w_lv", HID, LAT), ("w_dec", LAT, HID),
    ("wd_cat", HID, 2 * D_IN), ("wld_z", D_IN, D_IN), ("wld_h", D_IN, D_IN),
]
WOFF = {}
_o = 0
for _n, _p, _c in WSPEC:
    WOFF[_n] = (_o, _p, _c)
    _o += _c
WBLOB_COLS = _o

BSPEC = [("nblz", HID, 1), ("blh", HID, 1), ("b_mu", LAT, 1), ("b_lv", LAT, 1),
         ("b_dec", HID, 1), ("nbldz", D_IN, 1), ("bldh", D_IN, 1), ("probs", 128, P)]
BOFF = {}
_o = 0
for _n, _p, _c in BSPEC:
    BOFF[_n] = (_o, _p, _c)
    _o += _c
BBLOB_COLS = _o

_CACHE = {}


def _build():
    nc = bacc.Bacc("TRN2", debug=False, num_devices=NCORES)
    AF = mybir.ActivationFunctionType

    esc = nc.dram_tensor("esc", [128, KT * COLS], F32, kind="ExternalInput")
    xTp = nc.dram_tensor("xTp", [D_IN, P * COLS], BF16, kind="ExternalInput")
    eeTp = nc.dram_tensor("eeTp", [D_EMB, P * COLS], BF16, kind="ExternalInput")
    teTp = nc.dram_tensor("teTp", [D_EMB, P * COLS], BF16, kind="ExternalInput")
    epsT = nc.dram_tensor("epsT", [LAT, COLS], F32, kind="ExternalInput")
    wblob = nc.dram_tensor("wblob", [128, WBLOB_COLS], BF16, kind="ExternalInput")
    bblob = nc.dram_tensor("bblob", [128, BBLOB_COLS], F32, kind="ExternalInput")

    a_out = nc.dram_tensor("a_out", [128, KT * COLS], F32, kind="ExternalOutput")
    mu_out = nc.dram_tensor("mu_out", [LAT, COLS], F32, kind="ExternalOutput")
    lv_out = nc.dram_tensor("lv_out", [LAT, COLS], F32, kind="ExternalOutput")
    rec_out = nc.dram_tensor("rec_out", [D_IN, COLS], F32, kind="ExternalOutput")

    dis_dram = nc.dram_tensor("dis_dram", [1, COLS], F32)
    dis_full = nc.dram_tensor("dis_full", [N], F32, addr_space="Shared")

    with tile.TileContext(nc) as tc:
        with (
            tc.tile_pool(name="singles", bufs=1) as sg,
            tc.tile_pool(name="esc_in", bufs=4) as esc_p,
            tc.tile_pool(name="ys4", bufs=3) as ys4_p,
            tc.tile_pool(name="yd4", bufs=2) as yd4_p,
            tc.tile_pool(name="work", bufs=2) as wk,
            tc.tile_pool(name="gps", bufs=1, space="PSUM") as gps,
            tc.tile_pool(name="mps", bufs=1, space="PSUM") as mps,
            tc.tile_pool(name="dram", bufs=1, space="DRAM") as dr,
        ):
            # ---- small loads first (scalar ring, blob DMAs) ----
            wblob_t = sg.tile([128, WBLOB_COLS], BF16)
            nc.scalar.dma_start(out=wblob_t[:], in_=wblob[:])
            bblob_t = sg.tile([128, BBLOB_COLS], F32)
            nc.scalar.dma_start(out=bblob_t[:], in_=bblob[:])

            def w(name):
                o, p, c = WOFF[name]
                return wblob_t[0:p, o:o + c]

            def b(name):
                o, p, c = BOFF[name]
                return bblob_t[0:p, o:o + c]

            xT_t = sg.tile([D_IN, P * COLS], BF16)
            nc.scalar.dma_start(out=xT_t[:], in_=xTp[:])
            eeT_t = sg.tile([D_EMB, P * COLS], BF16)
            nc.scalar.dma_start(out=eeT_t[:], in_=eeTp[:])
            teT_t = sg.tile([D_EMB, P * COLS], BF16)
            nc.scalar.dma_start(out=teT_t[:], in_=teTp[:])
            epsT_t = sg.tile([LAT, COLS], F32)
            nc.scalar.dma_start(out=epsT_t[:], in_=epsT[:])
            ones_t = sg.tile([128, 1], F32)
            nc.vector.memset(ones_t[:], 1.0)
            ones_row = sg.tile([1, 128], F32)
            nc.vector.memset(ones_row[:], 1.0)

            # ---- stream: esc -> sigmoid (in place) -> deg (f32) ----
            deg_ps = mps.tile([1, COLS], F32, tag="uda", name="deg_ps")
            esc_ts = []
            W = GSIZE * COLS
            for g in range(NG):
                esc_t = esc_p.tile([128, W], F32, name="esc_t")
                nc.sync.dma_start(out=esc_t[:], in_=esc[:, g * W:(g + 1) * W])
                nc.scalar.activation(out=esc_t[:], in_=esc_t[:], func=AF.Sigmoid)
                esc_ts.append(esc_t)
                for kk in range(GSIZE):
                    nc.tensor.matmul(
                        deg_ps[:], ones_t[:], esc_t[:, kk * COLS:(kk + 1) * COLS],
                        start=(g == 0 and kk == 0), stop=(g == NG - 1 and kk == GSIZE - 1),
                    )

            # ---- ent/tim relu features (feat-major, bf16) ----
            ent_t = sg.tile([HID, P * COLS], BF16)
            tim_t = sg.tile([HID, P * COLS], BF16)
            for p in range(P):
                psl = slice(p * COLS, (p + 1) * COLS)
                ps1 = gps.tile([HID, COLS], F32, tag="g2", name="ent_ps")
                nc.tensor.matmul(ps1[:], w("w_ent"), eeT_t[:, psl], start=True, stop=True)
                nc.scalar.activation(out=ent_t[:, psl], in_=ps1[:], func=AF.Relu)
                ps2 = gps.tile([HID, COLS], F32, tag="g3", name="tim_ps")
                nc.tensor.matmul(ps2[:], w("w_time"), teT_t[:, psl], start=True, stop=True)
                nc.scalar.activation(out=tim_t[:, psl], in_=ps2[:], func=AF.Relu)

            # ---- local Y shard (node-major, unscaled), early AllGather ----
            ag1_in = dr.tile([COLS, YF], BF16)
            ag1_out = dr.tile([N, YF], BF16, addr_space="Shared")
            MT = COLS // 128
            for p in range(P):
                for m in range(MT):
                    msl = slice(m * 128, (m + 1) * 128)
                    psl = slice(p * COLS, (p + 1) * COLS)
                    y_ps = gps.tile([128, 2 * HID], F32, tag=f"g{m % 2}", name="y_ps")
                    nc.tensor.matmul(y_ps[:], xT_t[:, psl][:, msl], w("wc_x"),
                                     start=True, stop=False)
                    nc.tensor.matmul(y_ps[:], ent_t[:, psl][:, msl], w("wc_e"),
                                     start=False, stop=False)
                    nc.tensor.matmul(y_ps[:], tim_t[:, psl][:, msl], w("wc_t"),
                                     start=False, stop=True)
                    ysc = wk.tile([128, 2 * HID], BF16, name="ysc")
                    nc.vector.tensor_copy(out=ysc[:], in_=y_ps[:])
                    nc.scalar.dma_start(
                        out=ag1_in[m * 128:(m + 1) * 128, p * 2 * HID:(p + 1) * 2 * HID],
                        in_=ysc[:])
            nc.gpsimd.collective_compute(
                "AllGather", mybir.AluOpType.bypass,
                ins=[ag1_in[:].opt()], outs=[ag1_out[:].opt()],
                replica_groups=[list(range(NCORES))],
            )

            # ---- dis = 1/sqrt(deg) broadcast; tiny dis AllGather ----
            deg_sb = sg.tile([1, COLS], F32)
            nc.vector.tensor_copy(out=deg_sb[:], in_=deg_ps[:])
            bc_ps = mps.tile([128, COLS], F32, tag="sp", name="bc_ps")
            nc.tensor.matmul(bc_ps[:], ones_row[:], deg_sb[:], start=True, stop=True)
            sq_bc = sg.tile([128, COLS], F32)
            nc.scalar.activation(out=sq_bc[:], in_=bc_ps[:], func=AF.Sqrt)
            dis_bc = sg.tile([128, COLS], F32)
            nc.vector.reciprocal(out=dis_bc[:], in_=sq_bc[:])
            nc.scalar.dma_start(out=dis_dram[:], in_=dis_bc[0:1, :])
            nc.gpsimd.collective_compute(
                "AllGather", mybir.AluOpType.bypass,
                ins=[dis_dram[:].opt()], outs=[dis_full[:].opt()],
                replica_groups=[list(range(NCORES))],
            )
            disf_nm = sg.tile([128, KT], F32)
            nc.scalar.dma_start(
                out=disf_nm[:], in_=dis_full[:].rearrange("(k p) -> p k", p=128))

            # ---- deferred: A writes + bf16 casts + dis_i scale in place ----
            a_bf = []
            for g in range(NG):
                nc.sync.dma_start(out=a_out[:, g * W:(g + 1) * W], in_=esc_ts[g][:])
                ab = sg.tile([128, W], BF16, name=f"a_bf{g}")
                nc.vector.tensor_copy(out=ab[:], in_=esc_ts[g][:])
                a_bf.append(ab)

            def a_tile(ki):
                return a_bf[ki // GSIZE][:, (ki % GSIZE) * COLS:(ki % GSIZE + 1) * COLS]

            for ki in range(KT):
                at = a_tile(ki)
                nc.vector.tensor_scalar_mul(at, at, disf_nm[:, ki:ki + 1])

            # ---- encoder big matmul ----
            g_ps = [gps.tile([128, COLS], F32, tag=f"g{ft}", name=f"g_ps{ft}")
                    for ft in range(6)]
            for kb in range(KT // KB):
                ys4 = ys4_p.tile([128, KB, YF], BF16, name="ys4")
                nc.scalar.dma_start(
                    out=ys4[:],
                    in_=ag1_out[kb * KB * 128:(kb + 1) * KB * 128, :]
                    .rearrange("(b p) f -> p b f", p=128))
                for kk in range(KB):
                    ki = kb * KB + kk
                    for ft in range(6):
                        nc.tensor.matmul(
                            g_ps[ft][:], ys4[:, kk, ft * 128:(ft + 1) * 128], a_tile(ki),
                            start=(ki == 0), stop=(ki == KT - 1))

            # ---- encoder gates + Henc ----
            henc_t = sg.tile([HID, COLS], F32)
            for p in range(P):
                gz_sc = wk.tile([128, COLS], BF16, name="gz_sc")
                nc.vector.tensor_mul(gz_sc[:], g_ps[2 * p][:], dis_bc[:])
                u_ps = mps.tile([128, COLS], F32, tag="uda", name="uz_ps")
                nc.tensor.matmul(u_ps[:], w("wl_z"), gz_sc[:], start=True, stop=True)
                zc_t = wk.tile([HID, COLS], F32, name="zc_t")
                nc.scalar.activation(out=zc_t[:], in_=u_ps[:], func=AF.Sigmoid,
                                     bias=b("nblz"), scale=-1.0)
                gh_sc = wk.tile([128, COLS], BF16, name="gh_sc")
                nc.vector.tensor_mul(gh_sc[:], g_ps[2 * p + 1][:], dis_bc[:])
                uh_ps = mps.tile([128, COLS], F32, tag="uda", name="uh_ps")
                nc.tensor.matmul(uh_ps[:], w("wl_h"), gh_sc[:], start=True, stop=True)
                ht_t = wk.tile([HID, COLS], F32, name="ht_t")
                nc.scalar.activation(out=ht_t[:], in_=uh_ps[:], func=AF.Tanh,
                                     bias=b("blh"))
                zh_t = wk.tile([HID, COLS], F32, name="zh_t")
                nc.vector.tensor_mul(zh_t[:], zc_t[:], ht_t[:])
                if p == 0:
                    nc.vector.tensor_scalar_mul(henc_t[:], zh_t[:], b("probs")[:, 0:1])
                else:
                    zhp_t = wk.tile([HID, COLS], F32, name="zhp_t")
                    nc.vector.tensor_scalar_mul(zhp_t[:], zh_t[:], b("probs")[:, p:p + 1])
                    nc.vector.tensor_add(henc_t[:], henc_t[:], zhp_t[:])

            # ---- latent head ----
            h_bf = sg.tile([HID, COLS], BF16)
            nc.scalar.activation(out=h_bf[:], in_=henc_t[:], func=AF.Relu)
            mu_ps = mps.tile([LAT, COLS], F32, tag="sp", name="mu_ps")
            nc.tensor.matmul(mu_ps[:], w("w_mu"), h_bf[:], start=True, stop=True)
            mu_t = sg.tile([LAT, COLS], F32)
            nc.vector.tensor_scalar_add(mu_t[:], mu_ps[:], b("b_mu"))
            nc.scalar.dma_start(out=mu_out[:], in_=mu_t[:])
            lv_ps = mps.tile([LAT, COLS], F32, tag="sp", name="lv_ps")
            nc.tensor.matmul(lv_ps[:], w("w_lv"), h_bf[:], start=True, stop=True)
            lv_t = sg.tile([LAT, COLS], F32)
            nc.vector.tensor_scalar_add(lv_t[:], lv_ps[:], b("b_lv"))
            nc.scalar.dma_start(out=lv_out[:], in_=lv_t[:])
            std_t = wk.tile([LAT, COLS], F32, name="std_t")
            nc.scalar.activation(out=std_t[:], in_=lv_t[:], func=AF.Exp, scale=0.5)
            es_t = wk.tile([LAT, COLS], F32, name="es_t")
            nc.vector.tensor_mul(es_t[:], epsT_t[:], std_t[:])
            z_bf = sg.tile([LAT, COLS], BF16)
            nc.vector.tensor_add(z_bf[:], mu_t[:], es_t[:])
            d_ps = mps.tile([HID, COLS], F32, tag="sp", name="d_ps")
            nc.tensor.matmul(d_ps[:], w("w_dec"), z_bf[:], start=True, stop=True)
            d_bf = sg.tile([HID, COLS], BF16)
            nc.vector.tensor_scalar_add(d_bf[:], d_ps[:], b("b_dec"))

            # ---- decoder Y shard (unscaled) + AllGather ----
            ag2_in = dr.tile([COLS, 2 * D_IN], BF16)
            ag2_out = dr.tile([N, 2 * D_IN], BF16, addr_space="Shared")
            for m in range(MT):
                yd_ps = gps.tile([128, 2 * D_IN], F32, tag=f"g{2 + m % 2}", name="yd_ps")
                nc.tensor.matmul(yd_ps[:], d_bf[:, m * 128:(m + 1) * 128], w("wd_cat"),
                                 start=True, stop=True)
                ydsc = wk.tile([128, 2 * D_IN], BF16, name="ydsc")
                nc.vector.tensor_copy(out=ydsc[:], in_=yd_ps[:])
                nc.scalar.dma_start(out=ag2_in[m * 128:(m + 1) * 128, :], in_=ydsc[:])
            nc.gpsimd.collective_compute(
                "AllGather", mybir.AluOpType.bypass,
                ins=[ag2_in[:].opt()], outs=[ag2_out[:].opt()],
                replica_groups=[list(range(NCORES))],
            )

            # ---- decoder big matmul (two M=64 gates) + gates + recon ----
            gdz_ps = gps.tile([D_IN, COLS], F32, tag="g0", name="gdz_ps")
            gdh_ps = gps.tile([D_IN, COLS], F32, tag="g1", name="gdh_ps")
            for kb in range(KT // KB):
                yd4 = yd4_p.tile([128, KB, 2 * D_IN], BF16, name="yd4")
                nc.scalar.dma_start(
                    out=yd4[:],
                    in_=ag2_out[kb * KB * 128:(kb + 1) * KB * 128, :]
                    .rearrange("(b p) f -> p b f", p=128))
                for kk in range(KB):
                    ki = kb * KB + kk
                    nc.tensor.matmul(gdz_ps[:], yd4[:, kk, 0:D_IN], a_tile(ki),
                                     start=(ki == 0), stop=(ki == KT - 1))
                    nc.tensor.matmul(gdh_ps[:], yd4[:, kk, D_IN:2 * D_IN], a_tile(ki),
                                     start=(ki == 0), stop=(ki == KT - 1))
            gdz_sc = wk.tile([D_IN, COLS], BF16, name="gdz_sc")
            nc.vector.tensor_mul(gdz_sc[:], gdz_ps[:], dis_bc[0:D_IN, :])
            uzd_ps = mps.tile([D_IN, COLS], F32, tag="uda", name="uzd_ps")
            nc.tensor.matmul(uzd_ps[:], w("wld_z"), gdz_sc[:], start=True, stop=True)
            zcd_t = wk.tile([D_IN, COLS], F32, name="zcd_t")
            nc.scalar.activation(out=zcd_t[:], in_=uzd_ps[:], func=AF.Sigmoid,
                                 bias=b("nbldz"), scale=-1.0)
            gdh_sc = wk.tile([D_IN, COLS], BF16, name="gdh_sc")
            nc.vector.tensor_mul(gdh_sc[:], gdh_ps[:], dis_bc[0:D_IN, :])
            uhd_ps = mps.tile([D_IN, COLS], F32, tag="uda", name="uhd_ps")
            nc.tensor.matmul(uhd_ps[:], w("wld_h"), gdh_sc[:], start=True, stop=True)
            htd_t = wk.tile([D_IN, COLS], F32, name="htd_t")
            nc.scalar.activation(out=htd_t[:], in_=uhd_ps[:], func=AF.Tanh,
                                 bias=b("bldh"))
            prod_t = wk.tile([D_IN, COLS], F32, name="prod_t")
            nc.vector.tensor_mul(prod_t[:], zcd_t[:], htd_t[:])
            rec_t = wk.tile([D_IN, COLS], F32, name="rec_t")
            nc.vector.tensor_scalar_max(rec_t[:], prod_t[:], 0.0)
            nc.scalar.dma_start(out=rec_out[:], in_=rec_t[:])

    nc.compile()
    return nc


def _get_nc():
    if "nc" not in _CACHE:
        _CACHE["nc"] = _build()
    return _CACHE["nc"]


def _eps():
    if "eps" not in _CACHE:
        import jax

        with jax.default_device(jax.devices("cpu")[0]):
            e = jax.random.normal(jax.random.key(42), (N, LAT), jax.numpy.float32)
        _CACHE["eps"] = np.asarray(e)
    return _CACHE["eps"]


def _np(v):
    return np.asarray(v, dtype=np.float32)


def _pack_T(arr_rc, feat):
    # (COLS, P, feat) -> (feat, P*COLS), period-major column blocks, bf16
    a = arr_rc.transpose(1, 2, 0)  # (P, feat, COLS)
    out = np.empty((feat, P * COLS), dtype=BF)
    for p in range(P):
        out[:, p * COLS:(p + 1) * COLS] = a[p].astype(BF)
    return out


def make_in_maps(x, entity_emb, time_emb, params):
    x = _np(x)
    ee = _np(entity_emb)
    te = _np(time_emb)
    p = params
    t1, td = p["t1"], p["td"]
    eps = _eps()

    wc = np.concatenate([_np(t1["Wc_z"]), _np(t1["Wc_h"])], 1)
    wvals = {
        "w_ent": _np(p["W_ent"]), "w_time": _np(p["W_time"]),
        "wc_x": wc[:D_IN], "wc_e": wc[D_IN:D_IN + HID], "wc_t": wc[D_IN + HID:],
        "wl_z": _np(t1["Wl_z"])[:HID], "wl_h": _np(t1["Wl_h"])[:HID],
        "w_mu": _np(p["W_mu"]), "w_lv": _np(p["W_lv"]), "w_dec": _np(p["W_dec"]),
        "wd_cat": np.concatenate([_np(td["Wc_z"]), _np(td["Wc_h"])], 1),
        "wld_z": _np(td["Wl_z"])[:D_IN], "wld_h": _np(td["Wl_h"])[:D_IN],
    }
    wblob = np.zeros((128, WBLOB_COLS), dtype=BF)
    for name, (o, pp, c) in WOFF.items():
        wblob[0:pp, o:o + c] = wvals[name].astype(BF)

    att = _np(p["att1"])
    pr = np.exp(att - att.max())
    pr = (pr / pr.sum()).astype(np.float32)
    bvals = {
        "nblz": -(_np(t1["bc_z"]) @ _np(t1["Wl_z"])[:HID] + _np(t1["bl_z"])).reshape(HID, 1),
        "blh": (_np(t1["bc_h"]) @ _np(t1["Wl_h"])[:HID] + _np(t1["bl_h"])).reshape(HID, 1),
        "b_mu": _np(p["b_mu"]).reshape(LAT, 1),
        "b_lv": _np(p["b_lv"]).reshape(LAT, 1),
        "b_dec": _np(p["b_dec"]).reshape(HID, 1),
        "nbldz": -(_np(td["bc_z"]) @ _np(td["Wl_z"])[:D_IN] + _np(td["bl_z"])).reshape(D_IN, 1),
        "bldh": (_np(td["bc_h"]) @ _np(td["Wl_h"])[:D_IN] + _np(td["bl_h"])).reshape(D_IN, 1),
        "probs": np.broadcast_to(pr, (128, P)),
    }
    bblob = np.zeros((128, BBLOB_COLS), dtype=np.float32)
    for name, (o, pp, c) in BOFF.items():
        bblob[0:pp, o:o + c] = bvals[name].astype(np.float32)

    es_full = _np(p["edge_score"])
    in_maps = []
    for c in range(NCORES):
        rc = slice(c * COLS, (c + 1) * COLS)
        blk = es_full[:, rc]  # (4096, 512)
        esc_tiled = np.ascontiguousarray(
            blk.reshape(KT, 128, COLS).transpose(1, 0, 2).reshape(128, KT * COLS))
        in_maps.append({
            "wblob": wblob, "bblob": bblob, "esc": esc_tiled,
            "xTp": _pack_T(x[rc], D_IN),
            "eeTp": _pack_T(ee[rc], D_EMB),
            "teTp": _pack_T(te[rc], D_EMB),
            "epsT": np.ascontiguousarray(eps[rc].T),
        })
    return in_maps


def assemble(results):
    a_blocks, mu_blocks, lv_blocks, rec_blocks = [], [], [], []
    for c in range(NCORES):
        r = results[c]
        a_blocks.append(
            r["a_out"].reshape(128, KT, COLS).transpose(1, 0, 2).reshape(N, COLS))
        mu_blocks.append(r["mu_out"].T)
        lv_blocks.append(r["lv_out"].T)
        rec_blocks.append(r["rec_out"].T)
    A = np.concatenate(a_blocks, axis=1)
    mu = np.concatenate(mu_blocks, axis=0)
    lv = np.concatenate(lv_blocks, axis=0)
    rec = np.concatenate(rec_blocks, axis=0)
    return rec, mu, lv, A


def kernel(x, entity_emb, time_emb, num_nodes, params):
    nc = _get_nc()
    in_maps = make_in_maps(x, entity_emb, time_emb, params)
    res = run_bass_kernel_spmd(nc, in_maps, list(range(NCORES)))
    return assemble(res.results)


# revision 7
# speedup vs baseline: 1.1372x; 1.0608x over previous
"""Bass/Trainium2 kernel for nn_CausalGraphVAE (GCN message passing VAE).

Sharding: adjacency columns (= AnT output rows) split across 8 cores.
Per core: sigmoid+deg over its (4096,512) column block of edge_score
(in-place sigmoid, deferred A writes), dis=1/sqrt(deg) via broadcast
matmul, tiny dis AllGather, dis_i folded into the resident bf16 a-tiles,
X-side matmuls sharded by node rows with an early unscaled bf16
AllGather, big matmul A[:,rc]^T @ Ys per core, gates/latent local
(H0=0 kills the R gate; gate biases folded on host), second AllGather
for the decoder, outputs gathered on host. DMA traffic is split across
the two HWDGE rings: sync carries the big edge_score stream + A writes,
scalar carries everything else.
"""
import sys

if "/opt/trn_rl_repo" not in sys.path:
    sys.path.insert(0, "/opt/trn_rl_repo")

import numpy as np
import ml_dtypes

import concourse.bass as bass
import concourse.tile as tile
from concourse import bacc, mybir
from concourse.bass_utils import run_bass_kernel_spmd

NCORES = 8
N = 4096
COLS = N // NCORES          # 512 adjacency columns per core
KT = N // 128               # 32 contraction k-tiles
GSIZE = 4                   # k-tiles per DMA group in the stream
NG = KT // GSIZE            # 4 stream groups
D_IN = 64
D_EMB = 128
HID = 128
LAT = 64
P = 3
YF = P * 2 * HID            # 768 encoder Y features (z|h per period)
KB = 4                      # k-tiles per lhsT DMA batch in big matmuls
F32 = mybir.dt.float32
BF16 = mybir.dt.bfloat16
BF = ml_dtypes.bfloat16

# weight blob layout: name -> (offset, partitions, cols)
WSPEC = [
    ("w_ent", D_EMB, HID), ("w_time", D_EMB, HID),
    ("wc_x", D_IN, 2 * HID), ("wc_e", HID, 2 * HID), ("wc_t", HID, 2 * HID),
    ("wl_z", HID, HID), ("wl_h", HID, HID),
    ("w_mu", HID, LAT), ("w_lv", HID, LAT), ("w_dec", LAT, HID),
    ("wd_cat", HID, 2 * D_IN), ("wld_z", D_IN, D_IN), ("wld_h", D_IN, D_IN),
]
WOFF = {}
_o = 0
for _n, _p, _c in WSPEC:
    WOFF[_n] = (_o, _p, _c)
    _o += _c
WBLOB_COLS = _o

BSPEC = [("nblz", HID, 1), ("blh", HID, 1), ("b_mu", LAT, 1), ("b_lv", LAT, 1),
         ("b_dec", HID, 1), ("nbldz", D_IN, 1), ("bldh", D_IN, 1), ("probs", 128, P)]
BOFF = {}
_o = 0
for _n, _p, _c in BSPEC:
    BOFF[_n] = (_o, _p, _c)
    _o += _c
BBLOB_COLS = _o

_CACHE = {}


def _build():
    nc = bacc.Bacc("TRN2", debug=False, num_devices=NCORES)
    AF = mybir.ActivationFunctionType

    esc = nc.dram_tensor("esc", [128, KT * COLS], F32, kind="ExternalInput")
    xTp = nc.dram_tensor("xTp", [D_IN, P * COLS], BF16, kind="ExternalInput")
    eeTp = nc.dram_tensor("eeTp", [D_EMB, P * COLS], BF16, kind="ExternalInput")
    teTp = nc.dram_tensor("teTp", [D_EMB, P * COLS], BF16, kind="ExternalInput")
    epsT = nc.dram_tensor("epsT", [LAT, COLS], F32, kind="ExternalInput")
    wblob = nc.dram_tensor("wblob", [128, WBLOB_COLS], BF16, kind="ExternalInput")
    bblob = nc.dram_tensor("bblob", [128, BBLOB_COLS], F32, kind="ExternalInput")

    a_out = nc.dram_tensor("a_out", [128, KT * COLS], F32, kind="ExternalOutput")
    mu_out = nc.dram_tensor("mu_out", [LAT, COLS], F32, kind="ExternalOutput")
    lv_out = nc.dram_tensor("lv_out", [LAT, COLS], F32, kind="ExternalOutput")
    rec_out = nc.dram_tensor("rec_out", [D_IN, COLS], F32, kind="ExternalOutput")

    dis_dram = nc.dram_tensor("dis_dram", [1, COLS], F32)

    with tile.TileContext(nc) as tc:
        with (
            tc.tile_pool(name="singles", bufs=1) as sg,
            tc.tile_pool(name="esc_in", bufs=4) as esc_p,
            tc.tile_pool(name="ys4", bufs=3) as ys4_p,
            tc.tile_pool(name="yd4", bufs=2) as yd4_p,
            tc.tile_pool(name="work", bufs=2) as wk,
            tc.tile_pool(name="gps", bufs=1, space="PSUM") as gps,
            tc.tile_pool(name="mps", bufs=1, space="PSUM") as mps,
            tc.tile_pool(name="dram", bufs=1, space="DRAM") as dr,
        ):
            # ---- small loads first (scalar ring, blob DMAs) ----
            wblob_t = sg.tile([128, WBLOB_COLS], BF16)
            nc.scalar.dma_start(out=wblob_t[:], in_=wblob[:])
            bblob_t = sg.tile([128, BBLOB_COLS], F32)
            nc.scalar.dma_start(out=bblob_t[:], in_=bblob[:])

            def w(name):
                o, p, c = WOFF[name]
                return wblob_t[0:p, o:o + c]

            def b(name):
                o, p, c = BOFF[name]
                return bblob_t[0:p, o:o + c]

            xT_t = sg.tile([D_IN, P * COLS], BF16)
            nc.scalar.dma_start(out=xT_t[:], in_=xTp[:])
            eeT_t = sg.tile([D_EMB, P * COLS], BF16)
            nc.scalar.dma_start(out=eeT_t[:], in_=eeTp[:])
            teT_t = sg.tile([D_EMB, P * COLS], BF16)
            nc.scalar.dma_start(out=teT_t[:], in_=teTp[:])
            epsT_t = sg.tile([LAT, COLS], F32)
            nc.scalar.dma_start(out=epsT_t[:], in_=epsT[:])
            ones_t = sg.tile([128, 1], BF16)
            nc.vector.memset(ones_t[:], 1.0)
            ones_row = sg.tile([1, 128], F32)
            nc.vector.memset(ones_row[:], 1.0)

            # ---- stream: esc -> sigmoid (in place) -> deg (f32) ----
            deg_ps = mps.tile([1, COLS], F32, tag="uda", name="deg_ps")
            esc_ts = []
            a_bf = []
            W = GSIZE * COLS
            for g in range(NG):
                esc_t = esc_p.tile([128, W], F32, name="esc_t")
                nc.sync.dma_start(out=esc_t[:], in_=esc[:, g * W:(g + 1) * W])
                nc.scalar.activation(out=esc_t[:], in_=esc_t[:], func=AF.Sigmoid)
                esc_ts.append(esc_t)
                ab = sg.tile([128, W], BF16, name=f"a_bf{g}")
                nc.vector.tensor_copy(out=ab[:], in_=esc_t[:])
                a_bf.append(ab)
                for kk in range(GSIZE):
                    nc.tensor.matmul(
                        deg_ps[:], ones_t[:], ab[:, kk * COLS:(kk + 1) * COLS],
                        start=(g == 0 and kk == 0), stop=(g == NG - 1 and kk == GSIZE - 1),
                    )

            def a_tile(ki):
                return a_bf[ki // GSIZE][:, (ki % GSIZE) * COLS:(ki % GSIZE + 1) * COLS]

            # ---- ent/tim relu features (feat-major, bf16) ----
            ent_t = sg.tile([HID, P * COLS], BF16)
            tim_t = sg.tile([HID, P * COLS], BF16)
            for p in range(P):
                psl = slice(p * COLS, (p + 1) * COLS)
                ps1 = gps.tile([HID, COLS], F32, tag="g2", name="ent_ps")
                nc.tensor.matmul(ps1[:], w("w_ent"), eeT_t[:, psl], start=True, stop=True)
                nc.scalar.activation(out=ent_t[:, psl], in_=ps1[:], func=AF.Relu)
                ps2 = gps.tile([HID, COLS], F32, tag="g3", name="tim_ps")
                nc.tensor.matmul(ps2[:], w("w_time"), teT_t[:, psl], start=True, stop=True)
                nc.scalar.activation(out=tim_t[:, psl], in_=ps2[:], func=AF.Relu)

            # ---- local Y shard (node-major, unscaled), early AllGather ----
            ag1_in = dr.tile([COLS, YF], BF16)
            ag1_out = dr.tile([N, YF], BF16, addr_space="Shared")
            MT = COLS // 128
            y_sb = []
            for p in range(P):
                for m in range(MT):
                    msl = slice(m * 128, (m + 1) * 128)
                    psl = slice(p * COLS, (p + 1) * COLS)
                    y_ps = gps.tile([128, 2 * HID], F32, tag=f"g{m % 2}", name="y_ps")
                    nc.tensor.matmul(y_ps[:], xT_t[:, psl][:, msl], w("wc_x"),
                                     start=True, stop=False)
                    nc.tensor.matmul(y_ps[:], ent_t[:, psl][:, msl], w("wc_e"),
                                     start=False, stop=False)
                    nc.tensor.matmul(y_ps[:], tim_t[:, psl][:, msl], w("wc_t"),
                                     start=False, stop=True)
                    ysb = sg.tile([128, 2 * HID], F32, name=f"ysb{p}_{m}")
                    nc.vector.tensor_copy(out=ysb[:], in_=y_ps[:])
                    y_sb.append((p, m, ysb))

            # ---- dis = 1/sqrt(deg) broadcast + node-major gather ----
            deg_sb = sg.tile([1, COLS], F32)
            nc.vector.tensor_copy(out=deg_sb[:], in_=deg_ps[:])
            bc_ps = mps.tile([128, COLS], F32, tag="sp", name="bc_ps")
            nc.tensor.matmul(bc_ps[:], ones_row[:], deg_sb[:], start=True, stop=True)
            sq_bc = sg.tile([128, COLS], F32)
            nc.scalar.activation(out=sq_bc[:], in_=bc_ps[:], func=AF.Sqrt)
            dis_bc = sg.tile([128, COLS], F32)
            rscr = sg.tile([128, COLS], F32)
            nc.vector.reciprocal_approx_accurate(out=dis_bc[:], in_=sq_bc[:], scratch=rscr[:])
            nc.scalar.dma_start(out=dis_dram[:], in_=dis_bc[0:1, :])
            dis_nm = sg.tile([128, MT], F32)
            nc.scalar.dma_start(
                out=dis_nm[:], in_=dis_dram[0, :].rearrange("(m p) -> p m", p=128))

            # ---- scale Y shard rows by dis_i, ship, AllGather ----
            for p, m, ysb in y_sb:
                ysc = wk.tile([128, 2 * HID], BF16, name="ysc")
                nc.vector.tensor_scalar_mul(ysc[:], ysb[:], dis_nm[:, m:m + 1])
                nc.scalar.dma_start(
                    out=ag1_in[m * 128:(m + 1) * 128, p * 2 * HID:(p + 1) * 2 * HID],
                    in_=ysc[:])
            nc.gpsimd.collective_compute(
                "AllGather", mybir.AluOpType.bypass,
                ins=[ag1_in[:].opt()], outs=[ag1_out[:].opt()],
                replica_groups=[list(range(NCORES))],
            )

            # ---- deferred A output writes ----
            for g in range(NG):
                nc.sync.dma_start(out=a_out[:, g * W:(g + 1) * W], in_=esc_ts[g][:])

            # ---- encoder big matmul ----
            g_ps = [gps.tile([128, COLS], F32, tag=f"g{ft}", name=f"g_ps{ft}")
                    for ft in range(6)]
            for kb in range(KT // KB):
                ys4 = ys4_p.tile([128, KB, YF], BF16, name="ys4")
                nc.scalar.dma_start(
                    out=ys4[:],
                    in_=ag1_out[kb * KB * 128:(kb + 1) * KB * 128, :]
                    .rearrange("(b p) f -> p b f", p=128))
                for kk in range(KB):
                    ki = kb * KB + kk
                    for ft in range(6):
                        nc.tensor.matmul(
                            g_ps[ft][:], ys4[:, kk, ft * 128:(ft + 1) * 128], a_tile(ki),
                            start=(ki == 0), stop=(ki == KT - 1))

            # ---- encoder gates + Henc ----
            henc_t = sg.tile([HID, COLS], F32)
            for p in range(P):
                gz_sc = wk.tile([128, COLS], BF16, name="gz_sc")
                nc.vector.tensor_mul(gz_sc[:], g_ps[2 * p][:], dis_bc[:])
                u_ps = mps.tile([128, COLS], F32, tag="uda", name="uz_ps")
                nc.tensor.matmul(u_ps[:], w("wl_z"), gz_sc[:], start=True, stop=True)
                zc_t = wk.tile([HID, COLS], F32, name="zc_t")
                nc.scalar.activation(out=zc_t[:], in_=u_ps[:], func=AF.Sigmoid,
                                     bias=b("nblz"), scale=-1.0)
                gh_sc = wk.tile([128, COLS], BF16, name="gh_sc")
                nc.vector.tensor_mul(gh_sc[:], g_ps[2 * p + 1][:], dis_bc[:])
                uh_ps = mps.tile([128, COLS], F32, tag="uda", name="uh_ps")
                nc.tensor.matmul(uh_ps[:], w("wl_h"), gh_sc[:], start=True, stop=True)
                ht_t = wk.tile([HID, COLS], F32, name="ht_t")
                nc.scalar.activation(out=ht_t[:], in_=uh_ps[:], func=AF.Tanh,
                                     bias=b("blh"))
                zh_t = wk.tile([HID, COLS], F32, name="zh_t")
                nc.vector.tensor_mul(zh_t[:], zc_t[:], ht_t[:])
                if p == 0:
                    nc.vector.tensor_scalar_mul(henc_t[:], zh_t[:], b("probs")[:, 0:1])
                else:
                    zhp_t = wk.tile([HID, COLS], F32, name="zhp_t")
                    nc.vector.tensor_scalar_mul(zhp_t[:], zh_t[:], b("probs")[:, p:p + 1])
                    nc.vector.tensor_add(henc_t[:], henc_t[:], zhp_t[:])

            # ---- latent head ----
            h_bf = sg.tile([HID, COLS], BF16)
            nc.scalar.activation(out=h_bf[:], in_=henc_t[:], func=AF.Relu)
            mu_ps = mps.tile([LAT, COLS], F32, tag="sp", name="mu_ps")
            nc.tensor.matmul(mu_ps[:], w("w_mu"), h_bf[:], start=True, stop=True)
            mu_t = sg.tile([LAT, COLS], F32)
            nc.vector.tensor_scalar_add(mu_t[:], mu_ps[:], b("b_mu"))
            nc.scalar.dma_start(out=mu_out[:], in_=mu_t[:])
            lv_ps = mps.tile([LAT, COLS], F32, tag="sp", name="lv_ps")
            nc.tensor.matmul(lv_ps[:], w("w_lv"), h_bf[:], start=True, stop=True)
            lv_t = sg.tile([LAT, COLS], F32)
            nc.vector.tensor_scalar_add(lv_t[:], lv_ps[:], b("b_lv"))
            nc.scalar.dma_start(out=lv_out[:], in_=lv_t[:])
            std_t = wk.tile([LAT, COLS], F32, name="std_t")
            nc.scalar.activation(out=std_t[:], in_=lv_t[:], func=AF.Exp, scale=0.5)
            es_t = wk.tile([LAT, COLS], F32, name="es_t")
            nc.vector.tensor_mul(es_t[:], epsT_t[:], std_t[:])
            z_bf = sg.tile([LAT, COLS], BF16)
            nc.vector.tensor_add(z_bf[:], mu_t[:], es_t[:])
            d_ps = mps.tile([HID, COLS], F32, tag="sp", name="d_ps")
            nc.tensor.matmul(d_ps[:], w("w_dec"), z_bf[:], start=True, stop=True)
            d_bf = sg.tile([HID, COLS], BF16)
            nc.vector.tensor_scalar_add(d_bf[:], d_ps[:], b("b_dec"))

            # ---- decoder Y shard (unscaled) + AllGather ----
            ag2_in = dr.tile([COLS, 2 * D_IN], BF16)
            ag2_out = dr.tile([N, 2 * D_IN], BF16, addr_space="Shared")
            for m in range(MT):
                yd_ps = gps.tile([128, 2 * D_IN], F32, tag=f"g{2 + m % 2}", name="yd_ps")
                nc.tensor.matmul(yd_ps[:], d_bf[:, m * 128:(m + 1) * 128], w("wd_cat"),
                                 start=True, stop=True)
                ydsc = wk.tile([128, 2 * D_IN], BF16, name="ydsc")
                nc.vector.tensor_scalar_mul(ydsc[:], yd_ps[:], dis_nm[:, m:m + 1])
                nc.scalar.dma_start(out=ag2_in[m * 128:(m + 1) * 128, :], in_=ydsc[:])
            nc.gpsimd.collective_compute(
                "AllGather", mybir.AluOpType.bypass,
                ins=[ag2_in[:].opt()], outs=[ag2_out[:].opt()],
                replica_groups=[list(range(NCORES))],
            )

            # ---- decoder big matmul (two M=64 gates) + gates + recon ----
            gdz_ps = gps.tile([D_IN, COLS], F32, tag="g0", name="gdz_ps")
            gdh_ps = gps.tile([D_IN, COLS], F32, tag="g1", name="gdh_ps")
            for kb in range(KT // KB):
                yd4 = yd4_p.tile([128, KB, 2 * D_IN], BF16, name="yd4")
                nc.scalar.dma_start(
                    out=yd4[:],
                    in_=ag2_out[kb * KB * 128:(kb + 1) * KB * 128, :]
                    .rearrange("(b p) f -> p b f", p=128))
                for kk in range(KB):
                    ki = kb * KB + kk
                    nc.tensor.matmul(gdz_ps[:], yd4[:, kk, 0:D_IN], a_tile(ki),
                                     start=(ki == 0), stop=(ki == KT - 1))
                    nc.tensor.matmul(gdh_ps[:], yd4[:, kk, D_IN:2 * D_IN], a_tile(ki),
                                     start=(ki == 0), stop=(ki == KT - 1))
            gdz_sc = wk.tile([D_IN, COLS], BF16, name="gdz_sc")
            nc.vector.tensor_mul(gdz_sc[:], gdz_ps[:], dis_bc[0:D_IN, :])
            uzd_ps = mps.tile([D_IN, COLS], F32, tag="uda", name="uzd_ps")
            nc.tensor.matmul(uzd_ps[:], w("wld_z"), gdz_sc[:], start=True, stop=True)
            zcd_t = wk.tile([D_IN, COLS], F32, name="zcd_t")
            nc.scalar.activation(out=zcd_t[:], in_=uzd_ps[:], func=AF.Sigmoid,
                                 bias=b("nbldz"), scale=-1.0)
            gdh_sc = wk.tile([D_IN, COLS], BF16, name="gdh_sc")
            nc.vector.tensor_mul(gdh_sc[:], gdh_ps[:], dis_bc[0:D_IN, :])
            uhd_ps = mps.tile([D_IN, COLS], F32, tag="uda", name="uhd_ps")
            nc.tensor.matmul(uhd_ps[:], w("wld_h"), gdh_sc[:], start=True, stop=True)
            htd_t = wk.tile([D_IN, COLS], F32, name="htd_t")
            nc.scalar.activation(out=htd_t[:], in_=uhd_ps[:], func=AF.Tanh,
                                 bias=b("bldh"))
            prod_t = wk.tile([D_IN, COLS], F32, name="prod_t")
            nc.vector.tensor_mul(prod_t[:], zcd_t[:], htd_t[:])
            rec_t = wk.tile([D_IN, COLS], F32, name="rec_t")
            nc.vector.tensor_scalar_max(rec_t[:], prod_t[:], 0.0)
            nc.scalar.dma_start(out=rec_out[:], in_=rec_t[:])

    nc.compile()
    return nc


def _get_nc():
    if "nc" not in _CACHE:
        _CACHE["nc"] = _build()
    return _CACHE["nc"]


def _eps():
    if "eps" not in _CACHE:
        import jax

        with jax.default_device(jax.devices("cpu")[0]):
            e = jax.random.normal(jax.random.key(42), (N, LAT), jax.numpy.float32)
        _CACHE["eps"] = np.asarray(e)
    return _CACHE["eps"]


def _np(v):
    return np.asarray(v, dtype=np.float32)


def _pack_T(arr_rc, feat):
    # (COLS, P, feat) -> (feat, P*COLS), period-major column blocks, bf16
    a = arr_rc.transpose(1, 2, 0)  # (P, feat, COLS)
    out = np.empty((feat, P * COLS), dtype=BF)
    for p in range(P):
        out[:, p * COLS:(p + 1) * COLS] = a[p].astype(BF)
    return out


def make_in_maps(x, entity_emb, time_emb, params):
    x = _np(x)
    ee = _np(entity_emb)
    te = _np(time_emb)
    p = params
    t1, td = p["t1"], p["td"]
    eps = _eps()

    wc = np.concatenate([_np(t1["Wc_z"]), _np(t1["Wc_h"])], 1)
    wvals = {
        "w_ent": _np(p["W_ent"]), "w_time": _np(p["W_time"]),
        "wc_x": wc[:D_IN], "wc_e": wc[D_IN:D_IN + HID], "wc_t": wc[D_IN + HID:],
        "wl_z": _np(t1["Wl_z"])[:HID], "wl_h": _np(t1["Wl_h"])[:HID],
        "w_mu": _np(p["W_mu"]), "w_lv": _np(p["W_lv"]), "w_dec": _np(p["W_dec"]),
        "wd_cat": np.concatenate([_np(td["Wc_z"]), _np(td["Wc_h"])], 1),
        "wld_z": _np(td["Wl_z"])[:D_IN], "wld_h": _np(td["Wl_h"])[:D_IN],
    }
    wblob = np.zeros((128, WBLOB_COLS), dtype=BF)
    for name, (o, pp, c) in WOFF.items():
        wblob[0:pp, o:o + c] = wvals[name].astype(BF)

    att = _np(p["att1"])
    pr = np.exp(att - att.max())
    pr = (pr / pr.sum()).astype(np.float32)
    bvals = {
        "nblz": -(_np(t1["bc_z"]) @ _np(t1["Wl_z"])[:HID] + _np(t1["bl_z"])).reshape(HID, 1),
        "blh": (_np(t1["bc_h"]) @ _np(t1["Wl_h"])[:HID] + _np(t1["bl_h"])).reshape(HID, 1),
        "b_mu": _np(p["b_mu"]).reshape(LAT, 1),
        "b_lv": _np(p["b_lv"]).reshape(LAT, 1),
        "b_dec": _np(p["b_dec"]).reshape(HID, 1),
        "nbldz": -(_np(td["bc_z"]) @ _np(td["Wl_z"])[:D_IN] + _np(td["bl_z"])).reshape(D_IN, 1),
        "bldh": (_np(td["bc_h"]) @ _np(td["Wl_h"])[:D_IN] + _np(td["bl_h"])).reshape(D_IN, 1),
        "probs": np.broadcast_to(pr, (128, P)),
    }
    bblob = np.zeros((128, BBLOB_COLS), dtype=np.float32)
    for name, (o, pp, c) in BOFF.items():
        bblob[0:pp, o:o + c] = bvals[name].astype(np.float32)

    es_full = _np(p["edge_score"])
    in_maps = []
    for c in range(NCORES):
        rc = slice(c * COLS, (c + 1) * COLS)
        blk = es_full[:, rc]  # (4096, 512)
        esc_tiled = np.ascontiguousarray(
            blk.reshape(KT, 128, COLS).transpose(1, 0, 2).reshape(128, KT * COLS))
        in_maps.append({
            "wblob": wblob, "bblob": bblob, "esc": esc_tiled,
            "xTp": _pack_T(x[rc], D_IN),
            "eeTp": _pack_T(ee[rc], D_EMB),
            "teTp": _pack_T(te[rc], D_EMB),
            "epsT": np.ascontiguousarray(eps[rc].T),
        })
    return in_maps


def assemble(results):
    a_blocks, mu_blocks, lv_blocks, rec_blocks = [], [], [], []
    for c in range(NCORES):
        r = results[c]
        a_blocks.append(
            r["a_out"].reshape(128, KT, COLS).transpose(1, 0, 2).reshape(N, COLS))
        mu_blocks.append(r["mu_out"].T)
        lv_blocks.append(r["lv_out"].T)
        rec_blocks.append(r["rec_out"].T)
    A = np.concatenate(a_blocks, axis=1)
    mu = np.concatenate(mu_blocks, axis=0)
    lv = np.concatenate(lv_blocks, axis=0)
    rec = np.concatenate(rec_blocks, axis=0)
    return rec, mu, lv, A


def kernel(x, entity_emb, time_emb, num_nodes, params):
    nc = _get_nc()
    in_maps = make_in_maps(x, entity_emb, time_emb, params)
    res = run_bass_kernel_spmd(nc, in_maps, list(range(NCORES)))
    return assemble(res.results)
